# revision 17
# baseline (speedup 1.0000x reference)
"""Trainium2 Bass kernel for nn_ATOM_RNN (GAT-entity attention + GRU cell).

Data-parallel over 8 NeuronCores: batch 65536 -> 8192/core.
Feature-on-partition layout: activations stored [feat, batch]; stored [in,out]
weights are used directly as matmul lhsT. float32r (rounded fp32, 1 cyc/row
on PE at N>=256) for every matmul operand.

Key restructurings vs the reference graph:
  - Wh @ a[:64] == h_mix @ (W @ a[:64]): the [64,64] GAT matmul is folded
    into two 64->1 projections (host precomputes W@a).
  - e/softmax over the two 5x5 blocks is built with selection matmuls
    (E = Ssel.T @ S), denominators D = Gsum.T @ exp(leaky(E)), and
    1/D = exp(-ln D) on the ACT engine (no slow DVE reciprocal).
  - the 3-layer output MLP runs as block-diagonal matmuls over all 5
    attention rows at once.
  - GRU gates r,z computed by ONE K=128 matmul per subtile from xh=[x;h].

Engine partition rules honored: matmul operand base partition in {0,32,64}
with lhsT base == rhs base; f32r matmul outputs only at base 0; ACT/DVE
never shift partitions (all repacking via DMA).
"""
import sys
sys.path.insert(0, '/opt/trn_rl_repo')
import numpy as np

B = 65536
NCORES = 8
BL = B // NCORES          # 8192 per core
ALPHA = 0.01

_CACHE = {}


def _g(t):  # entity t -> input-proj weight group
    return 0 if t < 5 else (1 if t < 9 else 2)


def _constants(w):
    f32 = np.float32
    win = [w['w_in0'], w['w_in1'], w['w_in2']]
    bin_ = [w['b_in0'], w['b_in1'], w['b_in2']]

    # proj: 5 matmuls, rhs = obsx rows [0:32],[0:32],[32:64],[32:64],[64:80]
    # mm j covers entity pair (2j, 2j+1): out cols 0:64 -> e=2j, 64:128 -> 2j+1
    mm_rows = [(0, 32), (0, 32), (32, 64), (32, 64), (64, 80)]
    mm_ents = [(0, 1), (2, 3), (4, 5), (6, 7), (8, 9)]
    CBD = np.zeros((80, 5 * 128), f32)
    for j, ((r0, r1), ents) in enumerate(zip(mm_rows, mm_ents)):
        for half, t in enumerate(ents):
            fr = 8 * t - r0
            CBD[r0 + fr:r0 + fr + 8, 128 * j + 64 * half:128 * j + 64 * half + 64] = win[_g(t)]
    bP = np.zeros((128, 5), f32)
    for j, ents in enumerate(mm_ents):
        bP[0:64, j] = bin_[_g(ents[0])]
        bP[64:128, j] = bin_[_g(ents[1])]

    W64 = w['W'].astype(np.float64)
    a64 = w['a'].astype(np.float64)
    Wa = np.concatenate([W64 @ a64[:64], W64 @ a64[64:]], axis=1).astype(f32)
    CL = np.zeros((128, 5 * 20), f32)       # S accumulation lhsT per pair
    for j, ents in enumerate(mm_ents):
        for half, t in enumerate(ents):
            CL[64 * half:64 * half + 64, 20 * j + 2 * t] = Wa[:, 0]
            CL[64 * half:64 * half + 64, 20 * j + 2 * t + 1] = Wa[:, 1]

    CS = np.zeros((20, 50), f32)            # E = CS.T @ S
    for i in range(5):
        for j in range(5):
            CS[2 * i, 5 * i + j] += 1            # e0(i,j) = s1[i] + s2[5+j]
            CS[2 * (5 + j) + 1, 5 * i + j] += 1
            CS[2 * (5 + j), 25 + 5 * i + j] += 1  # e1(i,j) = s1[5+j] + s2[i]
            CS[2 * i + 1, 25 + 5 * i + j] += 1

    CG = np.zeros((50, 10), f32)            # D = CG.T @ X
    for i in range(5):
        for j in range(5):
            CG[5 * i + j, i] += 1
            CG[25 + 5 * i + j, 5 + j] += 1

    Bm = np.zeros((10, 50), f32)            # Rexp = Bm.T @ R
    for i in range(5):
        for j in range(5):
            Bm[i, 5 * i + j] = 1
            Bm[5 + j, 25 + 5 * i + j] = 1
    CB2 = np.zeros((42, 50), f32)           # duplicated at rows 0 / 32
    CB2[0:10] = Bm
    CB2[32:42] = Bm

    W1 = np.zeros((50, 160), f32)           # o-MLP layer1 block diag
    for i in range(5):
        for j in range(5):
            W1[5 * i + j, 32 * i:32 * i + 32] = w['w_o1'][j]
            W1[25 + 5 * i + j, 32 * i:32 * i + 32] = w['w_o1'][5 + j]
    CW1 = W1
    b1cat = np.tile(w['b_o1'], 5)
    b1a = b1cat[0:80].reshape(-1, 1).astype(f32)
    b1b = b1cat[80:160].reshape(-1, 1).astype(f32)

    W2 = np.zeros((160, 80), f32)
    for i in range(5):
        W2[32 * i:32 * i + 32, 16 * i:16 * i + 16] = w['w_o2']
    CW2a = W2[0:80].copy()
    CW2b = W2[80:160].copy()
    b2 = np.tile(w['b_o2'], 5).reshape(-1, 1).astype(f32)

    CW3 = np.zeros((80, 5), f32)
    for i in range(5):
        CW3[16 * i:16 * i + 16, i] = w['w_o3'][:, 0]
    b3 = np.full((69, 1), float(w['b_o3'][0]), f32)

    CON5 = np.zeros((69, 1), f32)
    for r in (0, 32, 64):
        CON5[r:r + 5] = 1.0
    CON1 = np.zeros((65, 5), f32)
    for r in (0, 32, 64):
        CON1[r] = 1.0

    perm = [(4 + r) % 80 for r in range(80)] + [80, 81, 82, 83, 84]
    CF1 = np.zeros((101, 64), f32)
    CF1[0:85] = w['w_fc1'][perm]
    CF1[96:101] = w['w_fc1'][85:90]
    bf1 = w['b_fc1'].reshape(-1, 1).astype(f32)

    wihT = w['w_ih'].T.astype(f32)          # [64, 192]
    whhT = w['w_hh'].T.astype(f32)
    CRZ = np.zeros((128, 128), f32)         # [r|z] from xh=[x;h]
    CRZ[0:64, 0:64] = wihT[:, 0:64]
    CRZ[0:64, 64:128] = wihT[:, 64:128]
    CRZ[64:128, 0:64] = whhT[:, 0:64]
    CRZ[64:128, 64:128] = whhT[:, 64:128]
    brz = np.concatenate([w['b_ih'][0:64] + w['b_hh'][0:64],
                          w['b_ih'][64:128] + w['b_hh'][64:128]]
                         ).reshape(-1, 1).astype(f32)
    CNI = wihT[:, 128:192].copy()           # gin lhsT [64,64], rhs xh[0:64]
    CNH = np.zeros((128, 64), f32)          # ghn lhsT at base 64
    CNH[64:128] = whhT[:, 128:192]
    bihn = w['b_ih'][128:192].reshape(-1, 1).astype(f32)
    bhhn = w['b_hh'][128:192].reshape(-1, 1).astype(f32)

    CF2 = w['w_fc2'].astype(f32)            # [64, 5]
    bq = w['b_fc2'].reshape(-1, 1).astype(f32)

    return dict(CBD=CBD, bP=bP, CL=CL, CS=CS, CG=CG, CB2=CB2, CW1=CW1,
                b1a=b1a, b1b=b1b, CW2a=CW2a, CW2b=CW2b, b2=b2, CW3=CW3, b3=b3,
                CON5=CON5, CON1=CON1, CF1=CF1, bf1=bf1,
                CRZ=CRZ, brz=brz, CNI=CNI, CNH=CNH, bihn=bihn, bhhn=bhhn,
                CF2=CF2, bq=bq)


def _build_program(cshapes):
    import concourse.bacc as bacc
    import concourse.mybir as mybir
    from concourse.tile import TileContext

    F32 = mybir.dt.float32
    F32R = mybir.dt.float32r
    AF = mybir.ActivationFunctionType
    OP = mybir.AluOpType

    # Restrict bacc's activation-table choices to one set per pass:
    # natural_log_exp_and_others covers ALL pass-A funcs (prelu/exp/ln/relu),
    # sigmoid_and_others covers pass B (sigmoid/tanh). Without this the
    # chooser alternates exp_and_others <-> natural_log and inserts ~14
    # ACT_TABLE_LOADs (1.3us each). Indices must be preserved, so unwanted
    # sets are emptied rather than removed.
    from concourse import hw_specs as _hs
    if not hasattr(bacc, '_orig_gat'):
        bacc._orig_gat = bacc.get_activation_tables
        _keep = {'natural_log_exp_and_others', 'sigmoid_and_others'}
        def _filtered(arch):
            t = bacc._orig_gat(arch)
            return {k: (v if k in _keep else set()) for k, v in t.items()}
        bacc.get_activation_tables = _filtered

    nc = bacc.Bacc(None, target_bir_lowering=False)

    obsx = nc.declare_dram_parameter("obsx", [96, BL], F32R, isOutput=False)
    hTD = nc.declare_dram_parameter("hTD", [64, BL], F32R, isOutput=False)
    qT = nc.declare_dram_parameter("qT", [5, BL], F32, isOutput=True)
    hoT = nc.declare_dram_parameter("hoT", [64, BL], F32R, isOutput=True)

    BIAS = {'bP', 'b1a', 'b1b', 'b2', 'b3', 'bf1', 'brz', 'bihn', 'bhhn', 'bq'}
    cpar = {}
    for name, shp in cshapes.items():
        dt = F32 if name in BIAS else F32R
        cpar[name] = nc.declare_dram_parameter(name, list(shp), dt,
                                               isOutput=False)

    mm_rows = [(0, 32), (0, 32), (32, 64), (32, 64), (64, 80)]
    # pack slots for the [5]- and [1]-row families (4 pairs per xt-block)
    slot = [(0, 0), (32, 0), (64, 0), (0, 1024)]

    with TileContext(nc) as tc:
        with tc.tile_pool(name="const", bufs=1) as cp, \
             tc.tile_pool(name="xh", bufs=8) as xhp:

            C = {}
            for name, shp in cshapes.items():
                dt = F32 if name in BIAS else F32R
                C[name] = cp.tile(list(shp), dt, tag=name, name='c_'+name)
                nc.sync.dma_start(out=C[name][:], in_=cpar[name][:])

            XH = []

            with tc.tile_pool(name="og", bufs=5) as ogp, \
                 tc.tile_pool(name="hp", bufs=3) as hpp, \
                 tc.tile_pool(name="sc", bufs=2) as scp, \
                 tc.tile_pool(name="xf", bufs=2) as xfp, \
                 tc.tile_pool(name="df", bufs=3) as dfp, \
                 tc.tile_pool(name="h1", bufs=2) as h1p, \
                 tc.tile_pool(name="h2", bufs=2) as h2p, \
                 tc.tile_pool(name="h3", bufs=4) as h3p, \
                 tc.tile_pool(name="pp", bufs=2, space="PSUM") as ppp, \
                 tc.tile_pool(name="ch", bufs=2, space="PSUM") as chp:
                for xt in range(2):          # 2 blocks x 4 pairs x 1024 cols
                    OGs = []
                    XEs = []
                    DPs = [None, None]
                    RPs = [None, None]
                    for uu in range(4):
                        u = 4 * xt + uu
                        OG = ogp.tile([101, 1024], F32R, tag="OG")
                        nc.sync.dma_start(out=OG[0:96, :],
                                          in_=obsx[:, 1024 * u:1024 * (u + 1)])
                        OGs.append(OG)
                        S2 = chp.tile([20, 1024], F32, tag="ch")
                        for j in range(5):
                            r0, r1 = mm_rows[j]
                            PP = ppp.tile([128, 1024], F32, tag="pp")
                            for h in range(2):
                                nc.tensor.matmul(
                                    PP[:, 512 * h:512 * (h + 1)],
                                    C['CBD'][r0:r1, 128 * j:128 * (j + 1)],
                                    OG[r0:r1, 512 * h:512 * (h + 1)],
                                    start=True, stop=True)
                            Hp = hpp.tile([128, 1024], F32R, tag="Hp")
                            nc.scalar.activation(Hp[:], PP[:], AF.Prelu,
                                                 bias=C['bP'][:, j:j + 1],
                                                 alpha=ALPHA)
                            for h in range(2):
                                nc.tensor.matmul(
                                    S2[:, 512 * h:512 * (h + 1)],
                                    C['CL'][:, 20 * j:20 * (j + 1)],
                                    Hp[:, 512 * h:512 * (h + 1)],
                                    start=(j == 0), stop=(j == 4))
                        SC = scp.tile([20, 1024], F32R, tag="SC")
                        nc.scalar.activation(SC[:], S2[:], AF.Copy)
                        E2 = chp.tile([50, 1024], F32, tag="ch")
                        for h in range(2):
                            nc.tensor.matmul(E2[:, 512 * h:512 * (h + 1)],
                                             C['CS'][:],
                                             SC[:, 512 * h:512 * (h + 1)],
                                             start=True, stop=True)
                        XP = xfp.tile([50, 1024], F32, tag="XP")
                        nc.scalar.activation(XP[:], E2[:], AF.Prelu,
                                             alpha=ALPHA)
                        XE = xfp.tile([50, 1024], F32R, tag="XE", bufs=4)
                        nc.scalar.activation(XE[:], XP[:], AF.Exp)
                        XEs.append(XE)
                        D2 = chp.tile([10, 1024], F32, tag="ch")
                        for h in range(2):
                            nc.tensor.matmul(D2[:, 512 * h:512 * (h + 1)],
                                             C['CG'][:],
                                             XE[:, 512 * h:512 * (h + 1)],
                                             start=True, stop=True)
                        du, k = uu // 2, uu % 2
                        if k == 0:
                            DPs[du] = dfp.tile([42, 1024], F32, tag="DP", name="DPt")
                        nc.vector.tensor_copy(DPs[du][32 * k:32 * k + 10, :],
                                              D2[:])
                        if k == 1:
                            LD = dfp.tile([42, 1024], F32, tag="LD", bufs=1)
                            nc.scalar.activation(LD[:], DPs[du][:], AF.Ln)
                            RP = dfp.tile([42, 1024], F32R, tag="RP")
                            nc.scalar.activation(RP[:], LD[:], AF.Exp,
                                                 scale=-1.0)
                            RPs[du] = RP

                    # attention = X * (1/D broadcast)
                    for uu in range(4):
                        rb2 = 32 * (uu % 2)
                        RP = RPs[uu // 2]
                        RE = chp.tile([50, 1024], F32, tag="ch")
                        for h in range(2):
                            nc.tensor.matmul(
                                RE[:, 512 * h:512 * (h + 1)],
                                C['CB2'][rb2:rb2 + 10, :],
                                RP[rb2:rb2 + 10, 512 * h:512 * (h + 1)],
                                start=True, stop=True)
                        nc.vector.tensor_tensor(XEs[uu][:],
                                                XEs[uu][:].bitcast(F32),
                                                RE[:], OP.mult)

                    # o-MLP (3 leaky layers, block diagonal over the 5 rows)
                    H3P = h3p.tile([69, 2048], F32, tag="h3")
                    for uu in range(4):
                        att = XEs[uu]
                        P1a = chp.tile([80, 1024], F32, tag="ch")
                        P1b = chp.tile([80, 1024], F32, tag="ch")
                        for h in range(2):
                            nc.tensor.matmul(P1a[:, 512 * h:512 * (h + 1)],
                                             C['CW1'][:, 0:80],
                                             att[:, 512 * h:512 * (h + 1)],
                                             start=True, stop=True)
                            nc.tensor.matmul(P1b[:, 512 * h:512 * (h + 1)],
                                             C['CW1'][:, 80:160],
                                             att[:, 512 * h:512 * (h + 1)],
                                             start=True, stop=True)
                        H1A = h1p.tile([80, 1024], F32R, tag="H1A")
                        H1B = h1p.tile([80, 1024], F32R, tag="H1B")
                        nc.scalar.activation(H1A[:], P1a[:], AF.Prelu,
                                             bias=C['b1a'][:], alpha=ALPHA)
                        nc.scalar.activation(H1B[:], P1b[:], AF.Prelu,
                                             bias=C['b1b'][:], alpha=ALPHA)
                        P2 = chp.tile([80, 1024], F32, tag="ch")
                        for h in range(2):
                            nc.tensor.matmul(P2[:, 512 * h:512 * (h + 1)],
                                             C['CW2a'][:],
                                             H1A[:, 512 * h:512 * (h + 1)],
                                             start=True, stop=False)
                            nc.tensor.matmul(P2[:, 512 * h:512 * (h + 1)],
                                             C['CW2b'][:],
                                             H1B[:, 512 * h:512 * (h + 1)],
                                             start=False, stop=True)
                        H2 = h2p.tile([80, 1024], F32R, tag="H2")
                        nc.scalar.activation(H2[:], P2[:], AF.Prelu,
                                             bias=C['b2'][:], alpha=ALPHA)
                        P3 = chp.tile([5, 1024], F32, tag="ch")
                        for h in range(2):
                            nc.tensor.matmul(P3[:, 512 * h:512 * (h + 1)],
                                             C['CW3'][:],
                                             H2[:, 512 * h:512 * (h + 1)],
                                             start=True, stop=True)
                        r3, f3 = slot[uu]
                        nc.scalar.activation(H3P[r3:r3 + 5, f3:f3 + 1024],
                                             P3[:], AF.Copy)
                    X3P = h3p.tile([69, 2048], F32R, tag="h3")
                    nc.scalar.activation(X3P[:], H3P[:], AF.Prelu,
                                         bias=C['b3'][:], alpha=ALPHA)
                    nc.scalar.activation(X3P[:], X3P[:].bitcast(F32), AF.Exp)

                    D3P = h3p.tile([65, 2048], F32, tag="h3")
                    for uu in range(4):
                        r3, f3 = slot[uu]
                        PD3 = chp.tile([1, 1024], F32, tag="ch")
                        for h in range(2):
                            nc.tensor.matmul(
                                PD3[:, 512 * h:512 * (h + 1)],
                                C['CON5'][r3:r3 + 5, :],
                                X3P[r3:r3 + 5, f3 + 512 * h:f3 + 512 * (h + 1)],
                                start=True, stop=True)
                        nc.vector.tensor_copy(D3P[r3:r3 + 1, f3:f3 + 1024],
                                              PD3[:])
                    L3 = h3p.tile([65, 2048], F32, tag="h3")
                    nc.scalar.activation(L3[:], D3P[:], AF.Ln)
                    R3P = h3p.tile([65, 2048], F32R, tag="h3")
                    nc.scalar.activation(R3P[:], L3[:], AF.Exp, scale=-1.0)

                    R5P = h3p.tile([69, 2048], F32, tag="h3")
                    for uu in range(4):
                        r3, f3 = slot[uu]
                        R5 = chp.tile([5, 1024], F32, tag="ch")
                        for h in range(2):
                            nc.tensor.matmul(
                                R5[:, 512 * h:512 * (h + 1)],
                                C['CON1'][r3:r3 + 1, :],
                                R3P[r3:r3 + 1, f3 + 512 * h:f3 + 512 * (h + 1)],
                                start=True, stop=True)
                        nc.vector.tensor_copy(R5P[r3:r3 + 5, f3:f3 + 1024],
                                              R5[:])
                    for uu in range(4):
                        r3, f3 = slot[uu]
                        nc.vector.tensor_tensor(
                            OGs[uu][96:101, :],
                            X3P[r3:r3 + 5, f3:f3 + 1024].bitcast(F32),
                            R5P[r3:r3 + 5, f3:f3 + 1024], OP.mult)

                    # fc1 + relu -> xh (x rows 0:64; h DMA'd into 64:128)
                    for uu in range(4):
                        u = 4 * xt + uu
                        r3, f3 = slot[uu]
                        PX = chp.tile([64, 1024], F32, tag="ch")
                        for h in range(2):
                            nc.tensor.matmul(PX[:, 512 * h:512 * (h + 1)],
                                             C['CF1'][:],
                                             OGs[uu][:, 512 * h:512 * (h + 1)],
                                             start=True, stop=True)
                        xh = xhp.tile([128, 1024], F32R, tag="xh")
                        nc.scalar.activation(xh[0:64, :], PX[:], AF.Relu,
                                             bias=C['bf1'][:])
                        nc.gpsimd.dma_start(out=xh[64:128, :],
                                            in_=hTD[:, 1024 * u:1024 * (u + 1)])
                        XH.append(xh)

            # ----- PASS B: GRU + fc2 (sigmoid/tanh activation table) -------
            with tc.tile_pool(name="gb", bufs=2) as gbp, \
                 tc.tile_pool(name="vt", bufs=3) as vtp, \
                 tc.tile_pool(name="ho", bufs=2) as hop, \
                 tc.tile_pool(name="gps", bufs=3, space="PSUM") as gps, \
                 tc.tile_pool(name="qps", bufs=1, space="PSUM") as qps:
                for u in range(8):
                    xh = XH[u]
                    PRZ = gps.tile([128, 1024], F32, tag="g")
                    for h in range(2):
                        nc.tensor.matmul(PRZ[:, 512 * h:512 * (h + 1)],
                                         C['CRZ'][:],
                                         xh[:, 512 * h:512 * (h + 1)],
                                         start=True, stop=True)
                    RZS = gbp.tile([128, 1024], F32, tag="RZS")
                    nc.scalar.activation(RZS[:], PRZ[:], AF.Sigmoid,
                                         bias=C['brz'][:])
                    Z64 = gbp.tile([64, 1024], F32, tag="Z64", bufs=1)
                    nc.gpsimd.dma_start(out=Z64[:], in_=RZS[64:128, :])
                    PGI = gps.tile([64, 1024], F32, tag="g")
                    PGH = gps.tile([64, 1024], F32, tag="g")
                    for h in range(2):
                        nc.tensor.matmul(PGI[:, 512 * h:512 * (h + 1)],
                                         C['CNI'][:],
                                         xh[0:64, 512 * h:512 * (h + 1)],
                                         start=True, stop=True)
                        nc.tensor.matmul(PGH[:, 512 * h:512 * (h + 1)],
                                         C['CNH'][64:128, :],
                                         xh[64:128, 512 * h:512 * (h + 1)],
                                         start=True, stop=True)
                    V1 = vtp.tile([64, 1024], F32, tag="vt")
                    nc.vector.scalar_tensor_tensor(V1[:], PGH[:], C['bhhn'][:],
                                                   RZS[0:64, :],
                                                   OP.add, OP.mult)
                    V3 = vtp.tile([64, 1024], F32, tag="vt")
                    nc.vector.tensor_tensor(V3[:], V1[:], PGI[:], OP.add)
                    N2 = gbp.tile([64, 1024], F32, tag="N2")
                    nc.scalar.activation(N2[:], V3[:], AF.Tanh,
                                         bias=C['bihn'][:])
                    H64 = gbp.tile([64, 1024], F32R, tag="H64", bufs=2)
                    nc.gpsimd.dma_start(out=H64[:],
                                        in_=hTD[:, 1024 * u:1024 * (u + 1)])
                    V4 = vtp.tile([64, 1024], F32, tag="vt")
                    nc.gpsimd.tensor_tensor(V4[:], H64[:].bitcast(F32),
                                            N2[:], OP.subtract)
                    V5 = vtp.tile([64, 1024], F32, tag="vt")
                    nc.vector.tensor_tensor(V5[:], Z64[:], V4[:], OP.mult)
                    HO = hop.tile([64, 1024], F32R, tag="HO")
                    nc.gpsimd.tensor_tensor(HO[:], N2[:], V5[:], OP.add)
                    nc.gpsimd.dma_start(out=hoT[:, 1024 * u:1024 * (u + 1)],
                                        in_=HO[:])
                    Q2 = qps.tile([5, 1024], F32, tag="q")
                    for h in range(2):
                        nc.tensor.matmul(Q2[:, 512 * h:512 * (h + 1)],
                                         C['CF2'][:],
                                         HO[:, 512 * h:512 * (h + 1)],
                                         start=True, stop=True)
                    QS = gbp.tile([5, 1024], F32, tag="QS", bufs=1)
                    nc.vector.tensor_scalar_add(QS[:], Q2[:], C['bq'][:])
                    nc.gpsimd.dma_start(out=qT[:, 1024 * u:1024 * (u + 1)],
                                        in_=QS[:])

    nc.compile()
    return nc


def kernel(**inputs):
    from concourse.bass_utils import run_bass_kernel_spmd

    obs = np.asarray(inputs['obs'], np.float32)
    hid = np.asarray(inputs['hidden_state'], np.float32)
    consts = _constants({k: np.asarray(v, np.float32)
                         for k, v in inputs.items()
                         if k not in ('obs', 'hidden_state')})

    if 'nc' not in _CACHE:
        _CACHE['nc'] = _build_program({k: v.shape for k, v in consts.items()})
    nc = _CACHE['nc']

    perm = [(4 + r) % 80 for r in range(80)] + [80, 81, 82, 83, 84]
    obsT = np.zeros((96, B), np.float32)                  # rows 85:96 stay 0
    obsT[0:85] = obs.T[perm]
    hT = np.ascontiguousarray(hid.T)                      # [64, B]

    in_maps = []
    for c in range(NCORES):
        cols = slice(c * BL, (c + 1) * BL)
        m = {'obsx': np.ascontiguousarray(obsT[:, cols]),
             'hTD': np.ascontiguousarray(hT[:, cols])}
        m.update(consts)
        in_maps.append(m)

    res = run_bass_kernel_spmd(nc, in_maps, list(range(NCORES)))

    q = np.empty((B, 5), np.float32)
    h = np.empty((B, 64), np.float32)
    for c, r in enumerate(res.results):
        cols = slice(c * BL, (c + 1) * BL)
        q[cols] = r['qT'].T
        h[cols] = r['hoT'].T
    return q, h


# revision 18
# speedup vs baseline: 1.0554x; 1.0554x over previous
"""Trainium2 Bass kernel for nn_ATOM_RNN (GAT-entity attention + GRU cell).

Data-parallel over 8 NeuronCores: batch 65536 -> 8192/core.
Feature-on-partition layout: activations stored [feat, batch]; stored [in,out]
weights are used directly as matmul lhsT. float32r (rounded fp32, 1 cyc/row
on PE at N>=256) for every matmul operand.

Key restructurings vs the reference graph:
  - Wh @ a[:64] == h_mix @ (W @ a[:64]): the [64,64] GAT matmul is folded
    into two 64->1 projections (host precomputes W@a).
  - e/softmax over the two 5x5 blocks is built with selection matmuls
    (E = Ssel.T @ S), denominators D = Gsum.T @ exp(leaky(E)), and
    1/D = exp(-ln D) on the ACT engine (no slow DVE reciprocal).
  - the 3-layer output MLP runs as block-diagonal matmuls over all 5
    attention rows at once.
  - GRU gates r,z computed by ONE K=128 matmul per subtile from xh=[x;h].

Engine partition rules honored: matmul operand base partition in {0,32,64}
with lhsT base == rhs base; f32r matmul outputs only at base 0; ACT/DVE
never shift partitions (all repacking via DMA).
"""
import sys
sys.path.insert(0, '/opt/trn_rl_repo')
import numpy as np

B = 65536
NCORES = 8
BL = B // NCORES          # 8192 per core
ALPHA = 0.01

_CACHE = {}


def _g(t):  # entity t -> input-proj weight group
    return 0 if t < 5 else (1 if t < 9 else 2)


def _constants(w):
    f32 = np.float32
    win = [w['w_in0'], w['w_in1'], w['w_in2']]
    bin_ = [w['b_in0'], w['b_in1'], w['b_in2']]

    # proj: 5 matmuls, rhs = obsx rows [0:32],[0:32],[32:64],[32:64],[64:80]
    # mm j covers entity pair (2j, 2j+1): out cols 0:64 -> e=2j, 64:128 -> 2j+1
    mm_rows = [(0, 32), (0, 32), (32, 64), (32, 64), (64, 80)]
    mm_ents = [(0, 1), (2, 3), (4, 5), (6, 7), (8, 9)]
    CBD = np.zeros((80, 5 * 128), f32)
    for j, ((r0, r1), ents) in enumerate(zip(mm_rows, mm_ents)):
        for half, t in enumerate(ents):
            fr = 8 * t - r0
            CBD[r0 + fr:r0 + fr + 8, 128 * j + 64 * half:128 * j + 64 * half + 64] = win[_g(t)]
    bP = np.zeros((128, 5), f32)
    for j, ents in enumerate(mm_ents):
        bP[0:64, j] = bin_[_g(ents[0])]
        bP[64:128, j] = bin_[_g(ents[1])]

    W64 = w['W'].astype(np.float64)
    a64 = w['a'].astype(np.float64)
    Wa = np.concatenate([W64 @ a64[:64], W64 @ a64[64:]], axis=1).astype(f32)
    CL = np.zeros((128, 5 * 20), f32)       # S accumulation lhsT per pair
    for j, ents in enumerate(mm_ents):
        for half, t in enumerate(ents):
            CL[64 * half:64 * half + 64, 20 * j + 2 * t] = Wa[:, 0]
            CL[64 * half:64 * half + 64, 20 * j + 2 * t + 1] = Wa[:, 1]

    CS = np.zeros((20, 50), f32)            # E = CS.T @ S
    for i in range(5):
        for j in range(5):
            CS[2 * i, 5 * i + j] += 1            # e0(i,j) = s1[i] + s2[5+j]
            CS[2 * (5 + j) + 1, 5 * i + j] += 1
            CS[2 * (5 + j), 25 + 5 * i + j] += 1  # e1(i,j) = s1[5+j] + s2[i]
            CS[2 * i + 1, 25 + 5 * i + j] += 1

    CG = np.zeros((50, 10), f32)            # D = CG.T @ X
    for i in range(5):
        for j in range(5):
            CG[5 * i + j, i] += 1
            CG[25 + 5 * i + j, 5 + j] += 1

    Bm = np.zeros((10, 50), f32)            # Rexp = Bm.T @ R
    for i in range(5):
        for j in range(5):
            Bm[i, 5 * i + j] = 1
            Bm[5 + j, 25 + 5 * i + j] = 1
    CB2 = np.zeros((42, 50), f32)           # duplicated at rows 0 / 32
    CB2[0:10] = Bm
    CB2[32:42] = Bm

    W1 = np.zeros((50, 160), f32)           # o-MLP layer1 block diag
    for i in range(5):
        for j in range(5):
            W1[5 * i + j, 32 * i:32 * i + 32] = w['w_o1'][j]
            W1[25 + 5 * i + j, 32 * i:32 * i + 32] = w['w_o1'][5 + j]
    CW1 = W1
    b1cat = np.tile(w['b_o1'], 5)
    b1a = b1cat[0:80].reshape(-1, 1).astype(f32)
    b1b = b1cat[80:160].reshape(-1, 1).astype(f32)

    W2 = np.zeros((160, 80), f32)
    for i in range(5):
        W2[32 * i:32 * i + 32, 16 * i:16 * i + 16] = w['w_o2']
    CW2a = W2[0:80].copy()
    CW2b = W2[80:160].copy()
    b2 = np.tile(w['b_o2'], 5).reshape(-1, 1).astype(f32)

    CW3 = np.zeros((80, 5), f32)
    for i in range(5):
        CW3[16 * i:16 * i + 16, i] = w['w_o3'][:, 0]
    b3 = np.full((69, 1), float(w['b_o3'][0]), f32)

    CON5 = np.zeros((69, 1), f32)
    for r in (0, 32, 64):
        CON5[r:r + 5] = 1.0
    CON1 = np.zeros((65, 5), f32)
    for r in (0, 32, 64):
        CON1[r] = 1.0

    perm = [(4 + r) % 80 for r in range(80)] + [80, 81, 82, 83, 84]
    CF1 = np.zeros((101, 64), f32)
    CF1[0:85] = w['w_fc1'][perm]
    CF1[96:101] = w['w_fc1'][85:90]
    bf1 = w['b_fc1'].reshape(-1, 1).astype(f32)

    wihT = w['w_ih'].T.astype(f32)          # [64, 192]
    whhT = w['w_hh'].T.astype(f32)
    CRZ = np.zeros((128, 128), f32)         # [r|z] from xh=[x;h]
    CRZ[0:64, 0:64] = wihT[:, 0:64]
    CRZ[0:64, 64:128] = wihT[:, 64:128]
    CRZ[64:128, 0:64] = whhT[:, 0:64]
    CRZ[64:128, 64:128] = whhT[:, 64:128]
    brz = np.concatenate([w['b_ih'][0:64] + w['b_hh'][0:64],
                          w['b_ih'][64:128] + w['b_hh'][64:128]]
                         ).reshape(-1, 1).astype(f32)
    CNI = wihT[:, 128:192].copy()           # gin lhsT [64,64], rhs xh[0:64]
    CNH = np.zeros((128, 64), f32)          # ghn lhsT at base 64
    CNH[64:128] = whhT[:, 128:192]
    bihn = w['b_ih'][128:192].reshape(-1, 1).astype(f32)
    bhhn = w['b_hh'][128:192].reshape(-1, 1).astype(f32)

    CF2 = w['w_fc2'].astype(f32)            # [64, 5]
    bq = w['b_fc2'].reshape(-1, 1).astype(f32)

    return dict(CBD=CBD, bP=bP, CL=CL, CS=CS, CG=CG, CB2=CB2, CW1=CW1,
                b1a=b1a, b1b=b1b, CW2a=CW2a, CW2b=CW2b, b2=b2, CW3=CW3, b3=b3,
                CON5=CON5, CON1=CON1, CF1=CF1, bf1=bf1,
                CRZ=CRZ, brz=brz, CNI=CNI, CNH=CNH, bihn=bihn, bhhn=bhhn,
                CF2=CF2, bq=bq)


def _build_program(cshapes):
    import concourse.bacc as bacc
    import concourse.mybir as mybir
    from concourse.tile import TileContext

    F32 = mybir.dt.float32
    F32R = mybir.dt.float32r
    AF = mybir.ActivationFunctionType
    OP = mybir.AluOpType

    # Restrict bacc's activation-table choices to one set per pass:
    # natural_log_exp_and_others covers ALL pass-A funcs (prelu/exp/ln/relu),
    # sigmoid_and_others covers pass B (sigmoid/tanh). Without this the
    # chooser alternates exp_and_others <-> natural_log and inserts ~14
    # ACT_TABLE_LOADs (1.3us each). Indices must be preserved, so unwanted
    # sets are emptied rather than removed.
    from concourse import hw_specs as _hs
    if not hasattr(bacc, '_orig_gat'):
        bacc._orig_gat = bacc.get_activation_tables
        _keep = {'natural_log_exp_and_others', 'sigmoid_and_others'}
        def _filtered(arch):
            t = bacc._orig_gat(arch)
            return {k: (v if k in _keep else set()) for k, v in t.items()}
        bacc.get_activation_tables = _filtered

    nc = bacc.Bacc(None, target_bir_lowering=False)

    obsx = nc.declare_dram_parameter("obsx", [96, BL], F32R, isOutput=False)
    hTD = nc.declare_dram_parameter("hTD", [64, BL], F32R, isOutput=False)
    qT = nc.declare_dram_parameter("qT", [5, BL], F32, isOutput=True)
    hoT = nc.declare_dram_parameter("hoT", [64, BL], F32R, isOutput=True)

    BIAS = {'bP', 'b1a', 'b1b', 'b2', 'b3', 'bf1', 'brz', 'bihn', 'bhhn', 'bq'}
    cpar = {}
    for name, shp in cshapes.items():
        dt = F32 if name in BIAS else F32R
        cpar[name] = nc.declare_dram_parameter(name, list(shp), dt,
                                               isOutput=False)

    mm_rows = [(0, 32), (0, 32), (32, 64), (32, 64), (64, 80)]
    # pack slots for the [5]- and [1]-row families (4 pairs per xt-block)
    slot = [(0, 0), (32, 0), (64, 0), (0, 1024)]

    with TileContext(nc) as tc:
        with tc.tile_pool(name="const", bufs=1) as cp, \
             tc.tile_pool(name="xh", bufs=8) as xhp:

            C = {}
            for name, shp in cshapes.items():
                dt = F32 if name in BIAS else F32R
                C[name] = cp.tile(list(shp), dt, tag=name, name='c_'+name)
                nc.sync.dma_start(out=C[name][:], in_=cpar[name][:])

            XH = []

            with tc.tile_pool(name="og", bufs=5) as ogp, \
                 tc.tile_pool(name="hp", bufs=3) as hpp, \
                 tc.tile_pool(name="sc", bufs=2) as scp, \
                 tc.tile_pool(name="xf", bufs=2) as xfp, \
                 tc.tile_pool(name="df", bufs=3) as dfp, \
                 tc.tile_pool(name="h1", bufs=2) as h1p, \
                 tc.tile_pool(name="h2", bufs=2) as h2p, \
                 tc.tile_pool(name="h3", bufs=4) as h3p, \
                 tc.tile_pool(name="pp", bufs=2, space="PSUM") as ppp, \
                 tc.tile_pool(name="ch", bufs=2, space="PSUM") as chp:
                for xt in range(2):          # 2 blocks x 4 pairs x 1024 cols
                    OGs = []
                    XEs = []
                    DPs = [None, None]
                    RPs = [None, None]
                    for uu in range(4):
                        u = 4 * xt + uu
                        OG = ogp.tile([101, 1024], F32R, tag="OG")
                        nc.sync.dma_start(out=OG[0:96, :],
                                          in_=obsx[:, 1024 * u:1024 * (u + 1)])
                        OGs.append(OG)
                        S2 = chp.tile([20, 1024], F32, tag="ch")
                        for j in range(5):
                            r0, r1 = mm_rows[j]
                            PP = ppp.tile([128, 1024], F32, tag="pp")
                            for h in range(2):
                                nc.tensor.matmul(
                                    PP[:, 512 * h:512 * (h + 1)],
                                    C['CBD'][r0:r1, 128 * j:128 * (j + 1)],
                                    OG[r0:r1, 512 * h:512 * (h + 1)],
                                    start=True, stop=True)
                            Hp = hpp.tile([128, 1024], F32R, tag="Hp")
                            nc.scalar.activation(Hp[:], PP[:], AF.Prelu,
                                                 bias=C['bP'][:, j:j + 1],
                                                 alpha=ALPHA)
                            for h in range(2):
                                nc.tensor.matmul(
                                    S2[:, 512 * h:512 * (h + 1)],
                                    C['CL'][:, 20 * j:20 * (j + 1)],
                                    Hp[:, 512 * h:512 * (h + 1)],
                                    start=(j == 0), stop=(j == 4))
                        SC = scp.tile([20, 1024], F32R, tag="SC")
                        nc.scalar.activation(SC[:], S2[:], AF.Copy)
                        E2 = chp.tile([50, 1024], F32, tag="ch")
                        for h in range(2):
                            nc.tensor.matmul(E2[:, 512 * h:512 * (h + 1)],
                                             C['CS'][:],
                                             SC[:, 512 * h:512 * (h + 1)],
                                             start=True, stop=True)
                        XP = xfp.tile([50, 1024], F32, tag="XP")
                        nc.scalar.activation(XP[:], E2[:], AF.Prelu,
                                             alpha=ALPHA)
                        XE = xfp.tile([50, 1024], F32R, tag="XE", bufs=4)
                        nc.scalar.activation(XE[:], XP[:], AF.Exp)
                        XEs.append(XE)
                        D2 = chp.tile([10, 1024], F32, tag="ch")
                        for h in range(2):
                            nc.tensor.matmul(D2[:, 512 * h:512 * (h + 1)],
                                             C['CG'][:],
                                             XE[:, 512 * h:512 * (h + 1)],
                                             start=True, stop=True)
                        du, k = uu // 2, uu % 2
                        if k == 0:
                            DPs[du] = dfp.tile([42, 1024], F32, tag="DP", name="DPt")
                        nc.vector.tensor_copy(DPs[du][32 * k:32 * k + 10, :],
                                              D2[:])
                        if k == 1:
                            LD = dfp.tile([42, 1024], F32, tag="LD", bufs=1)
                            nc.scalar.activation(LD[:], DPs[du][:], AF.Ln)
                            RP = dfp.tile([42, 1024], F32R, tag="RP")
                            nc.scalar.activation(RP[:], LD[:], AF.Exp,
                                                 scale=-1.0)
                            RPs[du] = RP

                    # attention = X * (1/D broadcast)
                    for uu in range(4):
                        rb2 = 32 * (uu % 2)
                        RP = RPs[uu // 2]
                        RE = chp.tile([50, 1024], F32, tag="ch")
                        for h in range(2):
                            nc.tensor.matmul(
                                RE[:, 512 * h:512 * (h + 1)],
                                C['CB2'][rb2:rb2 + 10, :],
                                RP[rb2:rb2 + 10, 512 * h:512 * (h + 1)],
                                start=True, stop=True)
                        nc.vector.tensor_tensor(XEs[uu][:],
                                                XEs[uu][:].bitcast(F32),
                                                RE[:], OP.mult)

                    # o-MLP (3 leaky layers, block diagonal over the 5 rows)
                    H3P = h3p.tile([69, 2048], F32, tag="h3")
                    for uu in range(4):
                        att = XEs[uu]
                        P1a = chp.tile([80, 1024], F32, tag="ch")
                        P1b = chp.tile([80, 1024], F32, tag="ch")
                        for h in range(2):
                            nc.tensor.matmul(P1a[:, 512 * h:512 * (h + 1)],
                                             C['CW1'][:, 0:80],
                                             att[:, 512 * h:512 * (h + 1)],
                                             start=True, stop=True)
                            nc.tensor.matmul(P1b[:, 512 * h:512 * (h + 1)],
                                             C['CW1'][:, 80:160],
                                             att[:, 512 * h:512 * (h + 1)],
                                             start=True, stop=True)
                        H1A = h1p.tile([80, 1024], F32R, tag="H1A")
                        H1B = h1p.tile([80, 1024], F32R, tag="H1B")
                        nc.scalar.activation(H1A[:], P1a[:], AF.Prelu,
                                             bias=C['b1a'][:], alpha=ALPHA)
                        nc.scalar.activation(H1B[:], P1b[:], AF.Prelu,
                                             bias=C['b1b'][:], alpha=ALPHA)
                        P2 = chp.tile([80, 1024], F32, tag="ch")
                        for h in range(2):
                            nc.tensor.matmul(P2[:, 512 * h:512 * (h + 1)],
                                             C['CW2a'][:],
                                             H1A[:, 512 * h:512 * (h + 1)],
                                             start=True, stop=False)
                            nc.tensor.matmul(P2[:, 512 * h:512 * (h + 1)],
                                             C['CW2b'][:],
                                             H1B[:, 512 * h:512 * (h + 1)],
                                             start=False, stop=True)
                        H2 = h2p.tile([80, 1024], F32R, tag="H2")
                        nc.scalar.activation(H2[:], P2[:], AF.Prelu,
                                             bias=C['b2'][:], alpha=ALPHA)
                        P3 = chp.tile([5, 1024], F32, tag="ch")
                        for h in range(2):
                            nc.tensor.matmul(P3[:, 512 * h:512 * (h + 1)],
                                             C['CW3'][:],
                                             H2[:, 512 * h:512 * (h + 1)],
                                             start=True, stop=True)
                        r3, f3 = slot[uu]
                        nc.scalar.activation(H3P[r3:r3 + 5, f3:f3 + 1024],
                                             P3[:], AF.Copy)
                    X3P = h3p.tile([69, 2048], F32R, tag="h3")
                    nc.scalar.activation(X3P[:], H3P[:], AF.Prelu,
                                         bias=C['b3'][:], alpha=ALPHA)
                    nc.scalar.activation(X3P[:], X3P[:].bitcast(F32), AF.Exp)

                    D3P = h3p.tile([65, 2048], F32, tag="h3")
                    for uu in range(4):
                        r3, f3 = slot[uu]
                        PD3 = chp.tile([1, 1024], F32, tag="ch")
                        for h in range(2):
                            nc.tensor.matmul(
                                PD3[:, 512 * h:512 * (h + 1)],
                                C['CON5'][r3:r3 + 5, :],
                                X3P[r3:r3 + 5, f3 + 512 * h:f3 + 512 * (h + 1)],
                                start=True, stop=True)
                        nc.vector.tensor_copy(D3P[r3:r3 + 1, f3:f3 + 1024],
                                              PD3[:])
                    L3 = h3p.tile([65, 2048], F32, tag="h3")
                    nc.scalar.activation(L3[:], D3P[:], AF.Ln)
                    R3P = h3p.tile([65, 2048], F32R, tag="h3")
                    nc.scalar.activation(R3P[:], L3[:], AF.Exp, scale=-1.0)

                    R5P = h3p.tile([69, 2048], F32, tag="h3")
                    for uu in range(4):
                        r3, f3 = slot[uu]
                        R5 = chp.tile([5, 1024], F32, tag="ch")
                        for h in range(2):
                            nc.tensor.matmul(
                                R5[:, 512 * h:512 * (h + 1)],
                                C['CON1'][r3:r3 + 1, :],
                                R3P[r3:r3 + 1, f3 + 512 * h:f3 + 512 * (h + 1)],
                                start=True, stop=True)
                        nc.vector.tensor_copy(R5P[r3:r3 + 5, f3:f3 + 1024],
                                              R5[:])
                    for uu in range(4):
                        r3, f3 = slot[uu]
                        nc.vector.tensor_tensor(
                            OGs[uu][96:101, :],
                            X3P[r3:r3 + 5, f3:f3 + 1024].bitcast(F32),
                            R5P[r3:r3 + 5, f3:f3 + 1024], OP.mult)

                    # fc1 + relu -> xh (x rows 0:64; h DMA'd into 64:128)
                    for uu in range(4):
                        u = 4 * xt + uu
                        r3, f3 = slot[uu]
                        PX = chp.tile([64, 1024], F32, tag="ch")
                        for h in range(2):
                            nc.tensor.matmul(PX[:, 512 * h:512 * (h + 1)],
                                             C['CF1'][:],
                                             OGs[uu][:, 512 * h:512 * (h + 1)],
                                             start=True, stop=True)
                        xh = xhp.tile([128, 1024], F32R, tag="xh")
                        nc.scalar.activation(xh[0:64, :], PX[:], AF.Relu,
                                             bias=C['bf1'][:])
                        nc.gpsimd.dma_start(out=xh[64:128, :],
                                            in_=hTD[:, 1024 * u:1024 * (u + 1)])
                        XH.append(xh)

            # ----- PASS B: GRU + fc2 (sigmoid/tanh activation table) -------
            with tc.tile_pool(name="gb", bufs=2) as gbp, \
                 tc.tile_pool(name="vt", bufs=3) as vtp, \
                 tc.tile_pool(name="ho", bufs=2) as hop, \
                 tc.tile_pool(name="gps", bufs=3, space="PSUM") as gps, \
                 tc.tile_pool(name="qps", bufs=1, space="PSUM") as qps:
                for u in range(8):
                    xh = XH[u]
                    PRZ = gps.tile([128, 1024], F32, tag="g")
                    for h in range(2):
                        nc.tensor.matmul(PRZ[:, 512 * h:512 * (h + 1)],
                                         C['CRZ'][:],
                                         xh[:, 512 * h:512 * (h + 1)],
                                         start=True, stop=True)
                    RZS = gbp.tile([128, 1024], F32, tag="RZS")
                    nc.scalar.activation(RZS[:], PRZ[:], AF.Sigmoid,
                                         bias=C['brz'][:])
                    Z64 = gbp.tile([64, 1024], F32, tag="Z64", bufs=1)
                    nc.gpsimd.dma_start(out=Z64[:], in_=RZS[64:128, :])
                    PGI = gps.tile([64, 1024], F32, tag="g")
                    PGH = gps.tile([64, 1024], F32, tag="g")
                    for h in range(2):
                        nc.tensor.matmul(PGI[:, 512 * h:512 * (h + 1)],
                                         C['CNI'][:],
                                         xh[0:64, 512 * h:512 * (h + 1)],
                                         start=True, stop=True)
                        nc.tensor.matmul(PGH[:, 512 * h:512 * (h + 1)],
                                         C['CNH'][64:128, :],
                                         xh[64:128, 512 * h:512 * (h + 1)],
                                         start=True, stop=True)
                    V1 = vtp.tile([64, 1024], F32, tag="vt")
                    nc.vector.scalar_tensor_tensor(V1[:], PGH[:], C['bhhn'][:],
                                                   RZS[0:64, :],
                                                   OP.add, OP.mult)
                    V3 = vtp.tile([64, 1024], F32, tag="vt")
                    nc.vector.tensor_tensor(V3[:], V1[:], PGI[:], OP.add)
                    N2 = gbp.tile([64, 1024], F32, tag="N2")
                    nc.scalar.activation(N2[:], V3[:], AF.Tanh,
                                         bias=C['bihn'][:])
                    H64 = gbp.tile([64, 1024], F32R, tag="H64", bufs=2)
                    nc.gpsimd.dma_start(out=H64[:],
                                        in_=hTD[:, 1024 * u:1024 * (u + 1)])
                    V4 = vtp.tile([64, 1024], F32, tag="vt")
                    nc.vector.tensor_tensor(V4[:], H64[:].bitcast(F32),
                                            N2[:], OP.subtract)
                    V5 = vtp.tile([64, 1024], F32, tag="vt")
                    nc.vector.tensor_tensor(V5[:], Z64[:], V4[:], OP.mult)
                    HO = hop.tile([64, 1024], F32R, tag="HO")
                    nc.vector.tensor_tensor(HO[:], N2[:], V5[:], OP.add)
                    nc.gpsimd.dma_start(out=hoT[:, 1024 * u:1024 * (u + 1)],
                                        in_=HO[:])
                    Q2 = qps.tile([5, 1024], F32, tag="q")
                    for h in range(2):
                        nc.tensor.matmul(Q2[:, 512 * h:512 * (h + 1)],
                                         C['CF2'][:],
                                         HO[:, 512 * h:512 * (h + 1)],
                                         start=True, stop=True)
                    QS = gbp.tile([5, 1024], F32, tag="QS", bufs=1)
                    nc.vector.tensor_scalar_add(QS[:], Q2[:], C['bq'][:])
                    nc.gpsimd.dma_start(out=qT[:, 1024 * u:1024 * (u + 1)],
                                        in_=QS[:])

    nc.compile()
    return nc


def kernel(**inputs):
    from concourse.bass_utils import run_bass_kernel_spmd

    obs = np.asarray(inputs['obs'], np.float32)
    hid = np.asarray(inputs['hidden_state'], np.float32)
    consts = _constants({k: np.asarray(v, np.float32)
                         for k, v in inputs.items()
                         if k not in ('obs', 'hidden_state')})

    if 'nc' not in _CACHE:
        _CACHE['nc'] = _build_program({k: v.shape for k, v in consts.items()})
    nc = _CACHE['nc']

    perm = [(4 + r) % 80 for r in range(80)] + [80, 81, 82, 83, 84]
    obsT = np.zeros((96, B), np.float32)                  # rows 85:96 stay 0
    obsT[0:85] = obs.T[perm]
    hT = np.ascontiguousarray(hid.T)                      # [64, B]

    in_maps = []
    for c in range(NCORES):
        cols = slice(c * BL, (c + 1) * BL)
        m = {'obsx': np.ascontiguousarray(obsT[:, cols]),
             'hTD': np.ascontiguousarray(hT[:, cols])}
        m.update(consts)
        in_maps.append(m)

    res = run_bass_kernel_spmd(nc, in_maps, list(range(NCORES)))

    q = np.empty((B, 5), np.float32)
    h = np.empty((B, 64), np.float32)
    for c, r in enumerate(res.results):
        cols = slice(c * BL, (c + 1) * BL)
        q[cols] = r['qT'].T
        h[cols] = r['hoT'].T
    return q, h


# revision 22
# speedup vs baseline: 1.1164x; 1.0578x over previous
"""Trainium2 Bass kernel for nn_ATOM_RNN (GAT-entity attention + GRU cell).

Data-parallel over 8 NeuronCores: batch 65536 -> 8192/core.
Feature-on-partition layout: activations stored [feat, batch]; stored [in,out]
weights are used directly as matmul lhsT. float32r (rounded fp32, 1 cyc/row
on PE at N>=256) for every matmul operand.

Key restructurings vs the reference graph:
  - Wh @ a[:64] == h_mix @ (W @ a[:64]): the [64,64] GAT matmul is folded
    into two 64->1 projections (host precomputes W@a).
  - e/softmax over the two 5x5 blocks is built with selection matmuls
    (E = Ssel.T @ S), denominators D = Gsum.T @ exp(leaky(E)), and
    1/D = exp(-ln D) on the ACT engine (no slow DVE reciprocal).
  - the 3-layer output MLP runs as block-diagonal matmuls over all 5
    attention rows at once.
  - GRU gates r,z computed by ONE K=128 matmul per subtile from xh=[x;h].

Engine partition rules honored: matmul operand base partition in {0,32,64}
with lhsT base == rhs base; f32r matmul outputs only at base 0; ACT/DVE
never shift partitions (all repacking via DMA).
"""
import sys
sys.path.insert(0, '/opt/trn_rl_repo')
import numpy as np

B = 65536
NCORES = 8
BL = B // NCORES          # 8192 per core
ALPHA = 0.01

_CACHE = {}


def _g(t):  # entity t -> input-proj weight group
    return 0 if t < 5 else (1 if t < 9 else 2)


def _constants(w):
    f32 = np.float32
    win = [w['w_in0'], w['w_in1'], w['w_in2']]
    bin_ = [w['b_in0'], w['b_in1'], w['b_in2']]

    # proj: 5 matmuls, rhs = obsx rows [0:32],[0:32],[32:64],[32:64],[64:80]
    # mm j covers entity pair (2j, 2j+1): out cols 0:64 -> e=2j, 64:128 -> 2j+1
    mm_rows = [(0, 32), (0, 32), (32, 64), (32, 64), (64, 80)]
    mm_ents = [(0, 1), (2, 3), (4, 5), (6, 7), (8, 9)]
    CBD = np.zeros((80, 5 * 128), f32)
    for j, ((r0, r1), ents) in enumerate(zip(mm_rows, mm_ents)):
        for half, t in enumerate(ents):
            fr = 8 * t - r0
            CBD[r0 + fr:r0 + fr + 8, 128 * j + 64 * half:128 * j + 64 * half + 64] = win[_g(t)]
    bP = np.zeros((128, 5), f32)
    for j, ents in enumerate(mm_ents):
        bP[0:64, j] = bin_[_g(ents[0])]
        bP[64:128, j] = bin_[_g(ents[1])]

    W64 = w['W'].astype(np.float64)
    a64 = w['a'].astype(np.float64)
    Wa = np.concatenate([W64 @ a64[:64], W64 @ a64[64:]], axis=1).astype(f32)
    CL = np.zeros((128, 5 * 20), f32)       # S accumulation lhsT per pair
    for j, ents in enumerate(mm_ents):
        for half, t in enumerate(ents):
            CL[64 * half:64 * half + 64, 20 * j + 2 * t] = Wa[:, 0]
            CL[64 * half:64 * half + 64, 20 * j + 2 * t + 1] = Wa[:, 1]

    CS = np.zeros((20, 50), f32)            # E = CS.T @ S
    for i in range(5):
        for j in range(5):
            CS[2 * i, 5 * i + j] += 1            # e0(i,j) = s1[i] + s2[5+j]
            CS[2 * (5 + j) + 1, 5 * i + j] += 1
            CS[2 * (5 + j), 25 + 5 * i + j] += 1  # e1(i,j) = s1[5+j] + s2[i]
            CS[2 * i + 1, 25 + 5 * i + j] += 1

    CG = np.zeros((50, 10), f32)            # D = CG.T @ X
    for i in range(5):
        for j in range(5):
            CG[5 * i + j, i] += 1
            CG[25 + 5 * i + j, 5 + j] += 1

    Bm = np.zeros((10, 50), f32)            # Rexp = Bm.T @ R
    for i in range(5):
        for j in range(5):
            Bm[i, 5 * i + j] = 1
            Bm[5 + j, 25 + 5 * i + j] = 1
    CB2 = np.zeros((42, 50), f32)           # duplicated at rows 0 / 32
    CB2[0:10] = Bm
    CB2[32:42] = Bm

    W1 = np.zeros((50, 160), f32)           # o-MLP layer1 block diag
    for i in range(5):
        for j in range(5):
            W1[5 * i + j, 32 * i:32 * i + 32] = w['w_o1'][j]
            W1[25 + 5 * i + j, 32 * i:32 * i + 32] = w['w_o1'][5 + j]
    CW1 = W1
    b1cat = np.tile(w['b_o1'], 5)
    b1a = b1cat[0:80].reshape(-1, 1).astype(f32)
    b1b = b1cat[80:160].reshape(-1, 1).astype(f32)

    W2 = np.zeros((160, 80), f32)
    for i in range(5):
        W2[32 * i:32 * i + 32, 16 * i:16 * i + 16] = w['w_o2']
    CW2a = W2[0:80].copy()
    CW2b = W2[80:160].copy()
    b2 = np.tile(w['b_o2'], 5).reshape(-1, 1).astype(f32)

    CW3 = np.zeros((80, 5), f32)
    for i in range(5):
        CW3[16 * i:16 * i + 16, i] = w['w_o3'][:, 0]
    b3 = np.full((69, 1), float(w['b_o3'][0]), f32)

    CON5 = np.zeros((69, 1), f32)
    for r in (0, 32, 64):
        CON5[r:r + 5] = 1.0
    CON1 = np.zeros((65, 5), f32)
    for r in (0, 32, 64):
        CON1[r] = 1.0

    perm = [(4 + r) % 80 for r in range(80)] + [80, 81, 82, 83, 84]
    CF1 = np.zeros((101, 64), f32)
    CF1[0:85] = w['w_fc1'][perm]
    CF1[96:101] = w['w_fc1'][85:90]
    bf1 = w['b_fc1'].reshape(-1, 1).astype(f32)

    wihT = w['w_ih'].T.astype(f32)          # [64, 192]
    whhT = w['w_hh'].T.astype(f32)
    CRZ = np.zeros((128, 128), f32)         # [r|z] from xh=[x;h]
    CRZ[0:64, 0:64] = wihT[:, 0:64]
    CRZ[0:64, 64:128] = wihT[:, 64:128]
    CRZ[64:128, 0:64] = whhT[:, 0:64]
    CRZ[64:128, 64:128] = whhT[:, 64:128]
    brz = np.concatenate([w['b_ih'][0:64] + w['b_hh'][0:64],
                          w['b_ih'][64:128] + w['b_hh'][64:128]]
                         ).reshape(-1, 1).astype(f32)
    CNI = wihT[:, 128:192].copy()           # gin lhsT [64,64], rhs xh[0:64]
    CNH = np.zeros((128, 64), f32)          # ghn lhsT at base 64
    CNH[64:128] = whhT[:, 128:192]
    bihn = w['b_ih'][128:192].reshape(-1, 1).astype(f32)
    bhhn = w['b_hh'][128:192].reshape(-1, 1).astype(f32)

    CF2 = w['w_fc2'].astype(f32)            # [64, 5]
    bq = w['b_fc2'].reshape(-1, 1).astype(f32)

    return dict(CBD=CBD, bP=bP, CL=CL, CS=CS, CG=CG, CB2=CB2, CW1=CW1,
                b1a=b1a, b1b=b1b, CW2a=CW2a, CW2b=CW2b, b2=b2, CW3=CW3, b3=b3,
                CON5=CON5, CON1=CON1, CF1=CF1, bf1=bf1,
                CRZ=CRZ, brz=brz, CNI=CNI, CNH=CNH, bihn=bihn, bhhn=bhhn,
                CF2=CF2, bq=bq)


def _build_program(cshapes):
    import concourse.bacc as bacc
    import concourse.mybir as mybir
    from concourse.tile import TileContext

    F32 = mybir.dt.float32
    F32R = mybir.dt.float32r
    AF = mybir.ActivationFunctionType
    OP = mybir.AluOpType

    # Restrict bacc's activation-table choices (see note in _constants caller):
    # natural_log_exp_and_others covers all attention-phase funcs, and
    # sigmoid_and_others covers the GRU phase; emptying the other sets stops
    # the chooser from thrashing between exp_and_others and natural_log.
    if not hasattr(bacc, '_orig_gat'):
        bacc._orig_gat = bacc.get_activation_tables
        _keep = {'natural_log_exp_and_others', 'sigmoid_and_others'}
        def _filtered(arch):
            t = bacc._orig_gat(arch)
            return {k: (v if k in _keep else set()) for k, v in t.items()}
        bacc.get_activation_tables = _filtered

    nc = bacc.Bacc(None, target_bir_lowering=False)

    obsx = nc.declare_dram_parameter("obsx", [96, BL], F32R, isOutput=False)
    hTD = nc.declare_dram_parameter("hTD", [64, BL], F32R, isOutput=False)
    qT = nc.declare_dram_parameter("qT", [5, BL], F32, isOutput=True)
    hoT = nc.declare_dram_parameter("hoT", [64, BL], F32R, isOutput=True)

    BIAS = {'bP', 'b1a', 'b1b', 'b2', 'b3', 'bf1', 'brz', 'bihn', 'bhhn', 'bq'}
    cpar = {}
    for name, shp in cshapes.items():
        dt = F32 if name in BIAS else F32R
        cpar[name] = nc.declare_dram_parameter(name, list(shp), dt,
                                               isOutput=False)

    mm_rows = [(0, 32), (0, 32), (32, 64), (32, 64), (64, 80)]
    slot = [(0, 0), (32, 0), (64, 0), (0, 1024)]

    with TileContext(nc) as tc:
        with tc.tile_pool(name="const", bufs=1) as cp, \
             tc.tile_pool(name="xh", bufs=5) as xhp, \
             tc.tile_pool(name="gb", bufs=2) as gbp, \
             tc.tile_pool(name="vt", bufs=3) as vtp, \
             tc.tile_pool(name="ho", bufs=2) as hop, \
             tc.tile_pool(name="og", bufs=3) as ogp, \
             tc.tile_pool(name="ogf", bufs=2) as ogfp, \
             tc.tile_pool(name="hp", bufs=2) as hpp, \
             tc.tile_pool(name="sc", bufs=2) as scp, \
             tc.tile_pool(name="xf", bufs=2) as xfp, \
             tc.tile_pool(name="df", bufs=2) as dfp, \
             tc.tile_pool(name="h1", bufs=1) as h1p, \
             tc.tile_pool(name="h2", bufs=2) as h2p, \
             tc.tile_pool(name="h3", bufs=3) as h3p, \
             tc.tile_pool(name="pp", bufs=2, space="PSUM") as ppp, \
             tc.tile_pool(name="ch", bufs=2, space="PSUM") as chp:

            C = {}
            for name, shp in cshapes.items():
                dt = F32 if name in BIAS else F32R
                C[name] = cp.tile(list(shp), dt, tag=name, name='c_'+name)
                nc.sync.dma_start(out=C[name][:], in_=cpar[name][:])

            XH = []

            def emit_gru(u):
                """GRU + fc2 for pair u (1024 cols). Sigmoid/tanh table."""
                xh = XH[u]
                PRZ = ppp.tile([128, 1024], F32, tag="pp", name="PRZ")
                for h in range(2):
                    nc.tensor.matmul(PRZ[:, 512 * h:512 * (h + 1)],
                                     C['CRZ'][:], xh[:, 512 * h:512 * (h + 1)],
                                     start=True, stop=True)
                RZS = gbp.tile([128, 1024], F32, tag="RZS", name="RZS")
                nc.scalar.activation(RZS[:], PRZ[:], AF.Sigmoid,
                                     bias=C['brz'][:])
                Z64 = gbp.tile([64, 1024], F32, tag="Z64", bufs=2, name="Z64")
                nc.gpsimd.dma_start(out=Z64[:], in_=RZS[64:128, :])
                PGI = ppp.tile([64, 1024], F32, tag="pp", name="PGI")
                PGH = ppp.tile([64, 1024], F32, tag="pp", name="PGH")
                for h in range(2):
                    nc.tensor.matmul(PGI[:, 512 * h:512 * (h + 1)],
                                     C['CNI'][:],
                                     xh[0:64, 512 * h:512 * (h + 1)],
                                     start=True, stop=True)
                    nc.tensor.matmul(PGH[:, 512 * h:512 * (h + 1)],
                                     C['CNH'][64:128, :],
                                     xh[64:128, 512 * h:512 * (h + 1)],
                                     start=True, stop=True)
                V1 = vtp.tile([64, 1024], F32, tag="vt", name="V1")
                nc.vector.scalar_tensor_tensor(V1[:], PGH[:], C['bhhn'][:],
                                               RZS[0:64, :], OP.add, OP.mult)
                V3 = vtp.tile([64, 1024], F32, tag="vt", name="V3")
                nc.vector.tensor_tensor(V3[:], V1[:], PGI[:], OP.add)
                N2 = gbp.tile([64, 1024], F32, tag="N2", name="N2")
                nc.scalar.activation(N2[:], V3[:], AF.Tanh, bias=C['bihn'][:])
                H64 = gbp.tile([64, 1024], F32R, tag="H64", bufs=2, name="H64")
                nc.gpsimd.dma_start(out=H64[:],
                                    in_=hTD[:, 1024 * u:1024 * (u + 1)])
                V4 = vtp.tile([64, 1024], F32, tag="vt", name="V4")
                nc.vector.tensor_tensor(V4[:], H64[:].bitcast(F32), N2[:],
                                        OP.subtract)
                V5 = vtp.tile([64, 1024], F32, tag="vt", name="V5")
                nc.vector.tensor_tensor(V5[:], Z64[:], V4[:], OP.mult)
                HO = hop.tile([64, 1024], F32R, tag="HO", name="HO")
                nc.vector.tensor_tensor(HO[:], N2[:], V5[:], OP.add)
                nc.gpsimd.dma_start(out=hoT[:, 1024 * u:1024 * (u + 1)],
                                    in_=HO[:])
                Q2 = ppp.tile([5, 1024], F32, tag="pp", name="Q2")
                for h in range(2):
                    nc.tensor.matmul(Q2[:, 512 * h:512 * (h + 1)], C['CF2'][:],
                                     HO[:, 512 * h:512 * (h + 1)],
                                     start=True, stop=True)
                QS = gbp.tile([5, 1024], F32, tag="QS", bufs=2, name="QS")
                nc.vector.tensor_scalar_add(QS[:], Q2[:], C['bq'][:])
                nc.gpsimd.dma_start(out=qT[:, 1024 * u:1024 * (u + 1)],
                                    in_=QS[:])

            for xt in range(2):          # 2 blocks x 4 pairs x 1024 cols
                XEs = []
                DPs = [None, None]
                RPs = [None, None]
                for uu in range(4):
                    u = 4 * xt + uu
                    OG = ogp.tile([80, 1024], F32R, tag="OG", name="OG")
                    nc.sync.dma_start(out=OG[:],
                                      in_=obsx[0:80, 1024 * u:1024 * (u + 1)])
                    S2 = chp.tile([20, 1024], F32, tag="ch", name="S2")
                    for j in range(5):
                        r0, r1 = mm_rows[j]
                        PP = ppp.tile([128, 1024], F32, tag="pp", name="PP")
                        for h in range(2):
                            nc.tensor.matmul(
                                PP[:, 512 * h:512 * (h + 1)],
                                C['CBD'][r0:r1, 128 * j:128 * (j + 1)],
                                OG[r0:r1, 512 * h:512 * (h + 1)],
                                start=True, stop=True)
                        Hp = hpp.tile([128, 1024], F32R, tag="Hp", name="Hp")
                        nc.scalar.activation(Hp[:], PP[:], AF.Prelu,
                                             bias=C['bP'][:, j:j + 1],
                                             alpha=ALPHA)
                        for h in range(2):
                            nc.tensor.matmul(
                                S2[:, 512 * h:512 * (h + 1)],
                                C['CL'][:, 20 * j:20 * (j + 1)],
                                Hp[:, 512 * h:512 * (h + 1)],
                                start=(j == 0), stop=(j == 4))
                    SC = scp.tile([20, 1024], F32R, tag="SC", name="SC")
                    nc.vector.tensor_copy(SC[:], S2[:])
                    E2 = chp.tile([50, 1024], F32, tag="ch", name="E2")
                    for h in range(2):
                        nc.tensor.matmul(E2[:, 512 * h:512 * (h + 1)],
                                         C['CS'][:],
                                         SC[:, 512 * h:512 * (h + 1)],
                                         start=True, stop=True)
                    XP = xfp.tile([50, 1024], F32, tag="XP", name="XP")
                    nc.scalar.activation(XP[:], E2[:], AF.Prelu, alpha=ALPHA)
                    XE = xfp.tile([50, 1024], F32R, tag="XE", bufs=4, name="XE")
                    nc.scalar.activation(XE[:], XP[:], AF.Exp)
                    XEs.append(XE)
                    D2 = chp.tile([10, 1024], F32, tag="ch", name="D2")
                    for h in range(2):
                        nc.tensor.matmul(D2[:, 512 * h:512 * (h + 1)],
                                         C['CG'][:],
                                         XE[:, 512 * h:512 * (h + 1)],
                                         start=True, stop=True)
                    du, k = uu // 2, uu % 2
                    if k == 0:
                        DPs[du] = dfp.tile([42, 1024], F32, tag="DP",
                                           name="DPt")
                    nc.vector.tensor_copy(DPs[du][32 * k:32 * k + 10, :],
                                          D2[:])
                    if k == 1:
                        LD = dfp.tile([42, 1024], F32, tag="LD", bufs=1,
                                      name="LD")
                        nc.scalar.activation(LD[:], DPs[du][:], AF.Ln)
                        RP = dfp.tile([42, 1024], F32R, tag="RP", name="RP")
                        nc.scalar.activation(RP[:], LD[:], AF.Exp, scale=-1.0)
                        RPs[du] = RP

                # attention = X * (1/D broadcast), in place over XE
                for uu in range(4):
                    rb2 = 32 * (uu % 2)
                    RP = RPs[uu // 2]
                    RE = chp.tile([50, 1024], F32, tag="ch", name="RE")
                    for h in range(2):
                        nc.tensor.matmul(
                            RE[:, 512 * h:512 * (h + 1)],
                            C['CB2'][rb2:rb2 + 10, :],
                            RP[rb2:rb2 + 10, 512 * h:512 * (h + 1)],
                            start=True, stop=True)
                    nc.vector.tensor_tensor(XEs[uu][:],
                                            XEs[uu][:].bitcast(F32),
                                            RE[:], OP.mult)

                # o-MLP (3 leaky layers, block diagonal over the 5 rows)
                H3P = h3p.tile([69, 2048], F32, tag="h3", name="H3P")
                for uu in range(4):
                    att = XEs[uu]
                    P1a = chp.tile([80, 1024], F32, tag="ch", name="P1a")
                    P1b = chp.tile([80, 1024], F32, tag="ch", name="P1b")
                    for h in range(2):
                        nc.tensor.matmul(P1a[:, 512 * h:512 * (h + 1)],
                                         C['CW1'][:, 0:80],
                                         att[:, 512 * h:512 * (h + 1)],
                                         start=True, stop=True)
                        nc.tensor.matmul(P1b[:, 512 * h:512 * (h + 1)],
                                         C['CW1'][:, 80:160],
                                         att[:, 512 * h:512 * (h + 1)],
                                         start=True, stop=True)
                    H1A = h1p.tile([80, 1024], F32R, tag="H1A", name="H1A")
                    H1B = h1p.tile([80, 1024], F32R, tag="H1B", name="H1B")
                    nc.scalar.activation(H1A[:], P1a[:], AF.Prelu,
                                         bias=C['b1a'][:], alpha=ALPHA)
                    nc.scalar.activation(H1B[:], P1b[:], AF.Prelu,
                                         bias=C['b1b'][:], alpha=ALPHA)
                    P2 = chp.tile([80, 1024], F32, tag="ch", name="P2")
                    for h in range(2):
                        nc.tensor.matmul(P2[:, 512 * h:512 * (h + 1)],
                                         C['CW2a'][:],
                                         H1A[:, 512 * h:512 * (h + 1)],
                                         start=True, stop=False)
                        nc.tensor.matmul(P2[:, 512 * h:512 * (h + 1)],
                                         C['CW2b'][:],
                                         H1B[:, 512 * h:512 * (h + 1)],
                                         start=False, stop=True)
                    H2 = h2p.tile([80, 1024], F32R, tag="H2", name="H2")
                    nc.scalar.activation(H2[:], P2[:], AF.Prelu,
                                         bias=C['b2'][:], alpha=ALPHA)
                    P3 = chp.tile([5, 1024], F32, tag="ch", name="P3")
                    for h in range(2):
                        nc.tensor.matmul(P3[:, 512 * h:512 * (h + 1)],
                                         C['CW3'][:],
                                         H2[:, 512 * h:512 * (h + 1)],
                                         start=True, stop=True)
                    r3, f3 = slot[uu]
                    nc.vector.tensor_copy(H3P[r3:r3 + 5, f3:f3 + 1024], P3[:])
                X3P = h3p.tile([69, 2048], F32R, tag="h3", name="X3P")
                nc.scalar.activation(X3P[:], H3P[:], AF.Prelu,
                                     bias=C['b3'][:], alpha=ALPHA)
                nc.scalar.activation(X3P[:], X3P[:].bitcast(F32), AF.Exp)

                D3P = h3p.tile([65, 2048], F32, tag="h3", name="D3P")
                for uu in range(4):
                    r3, f3 = slot[uu]
                    PD3 = chp.tile([1, 1024], F32, tag="ch", name="PD3")
                    for h in range(2):
                        nc.tensor.matmul(
                            PD3[:, 512 * h:512 * (h + 1)],
                            C['CON5'][r3:r3 + 5, :],
                            X3P[r3:r3 + 5, f3 + 512 * h:f3 + 512 * (h + 1)],
                            start=True, stop=True)
                    nc.vector.tensor_copy(D3P[r3:r3 + 1, f3:f3 + 1024],
                                          PD3[:])
                L3 = h3p.tile([65, 2048], F32, tag="h3", name="L3")
                nc.scalar.activation(L3[:], D3P[:], AF.Ln)
                R3P = h3p.tile([65, 2048], F32R, tag="h3", name="R3P")
                nc.scalar.activation(R3P[:], L3[:], AF.Exp, scale=-1.0)

                R5P = h3p.tile([69, 2048], F32, tag="h3", name="R5P")
                for uu in range(4):
                    r3, f3 = slot[uu]
                    R5 = chp.tile([5, 1024], F32, tag="ch", name="R5")
                    for h in range(2):
                        nc.tensor.matmul(
                            R5[:, 512 * h:512 * (h + 1)],
                            C['CON1'][r3:r3 + 1, :],
                            R3P[r3:r3 + 1, f3 + 512 * h:f3 + 512 * (h + 1)],
                            start=True, stop=True)
                    nc.vector.tensor_copy(R5P[r3:r3 + 5, f3:f3 + 1024],
                                          R5[:])

                # fc1 + relu -> xh (x rows 0:64; h DMA'd into 64:128)
                for uu in range(4):
                    u = 4 * xt + uu
                    r3, f3 = slot[uu]
                    OGF = ogfp.tile([101, 1024], F32R, tag="OGF", name="OGF")
                    nc.sync.dma_start(out=OGF[0:96, :],
                                      in_=obsx[:, 1024 * u:1024 * (u + 1)])
                    nc.vector.tensor_tensor(
                        OGF[96:101, :],
                        X3P[r3:r3 + 5, f3:f3 + 1024].bitcast(F32),
                        R5P[r3:r3 + 5, f3:f3 + 1024], OP.mult)
                    PX = chp.tile([64, 1024], F32, tag="ch", name="PX")
                    for h in range(2):
                        nc.tensor.matmul(PX[:, 512 * h:512 * (h + 1)],
                                         C['CF1'][:],
                                         OGF[:, 512 * h:512 * (h + 1)],
                                         start=True, stop=True)
                    xh = xhp.tile([128, 1024], F32R, tag="xh", name="xh")
                    nc.scalar.activation(xh[0:64, :], PX[:], AF.Relu,
                                         bias=C['bf1'][:])
                    nc.gpsimd.dma_start(out=xh[64:128, :],
                                        in_=hTD[:, 1024 * u:1024 * (u + 1)])
                    XH.append(xh)

                # GRU half-pass for this xt's pairs (overlaps next xt)
                for uu in range(4):
                    emit_gru(4 * xt + uu)

    nc.compile()
    return nc


def kernel(**inputs):
    from concourse.bass_utils import run_bass_kernel_spmd

    obs = np.asarray(inputs['obs'], np.float32)
    hid = np.asarray(inputs['hidden_state'], np.float32)
    consts = _constants({k: np.asarray(v, np.float32)
                         for k, v in inputs.items()
                         if k not in ('obs', 'hidden_state')})

    if 'nc' not in _CACHE:
        _CACHE['nc'] = _build_program({k: v.shape for k, v in consts.items()})
    nc = _CACHE['nc']

    perm = [(4 + r) % 80 for r in range(80)] + [80, 81, 82, 83, 84]
    obsT = np.zeros((96, B), np.float32)                  # rows 85:96 stay 0
    obsT[0:85] = obs.T[perm]
    hT = np.ascontiguousarray(hid.T)                      # [64, B]

    in_maps = []
    for c in range(NCORES):
        cols = slice(c * BL, (c + 1) * BL)
        m = {'obsx': np.ascontiguousarray(obsT[:, cols]),
             'hTD': np.ascontiguousarray(hT[:, cols])}
        m.update(consts)
        in_maps.append(m)

    res = run_bass_kernel_spmd(nc, in_maps, list(range(NCORES)))

    q = np.empty((B, 5), np.float32)
    h = np.empty((B, 64), np.float32)
    for c, r in enumerate(res.results):
        cols = slice(c * BL, (c + 1) * BL)
        q[cols] = r['qT'].T
        h[cols] = r['hoT'].T
    return q, h


# revision 23
# speedup vs baseline: 1.1203x; 1.0035x over previous
"""Trainium2 Bass kernel for nn_ATOM_RNN (GAT-entity attention + GRU cell).

Data-parallel over 8 NeuronCores: batch 65536 -> 8192/core.
Feature-on-partition layout: activations stored [feat, batch]; stored [in,out]
weights are used directly as matmul lhsT. float32r (rounded fp32, 1 cyc/row
on PE at N>=256) for every matmul operand.

Key restructurings vs the reference graph:
  - Wh @ a[:64] == h_mix @ (W @ a[:64]): the [64,64] GAT matmul is folded
    into two 64->1 projections (host precomputes W@a).
  - e/softmax over the two 5x5 blocks is built with selection matmuls
    (E = Ssel.T @ S), denominators D = Gsum.T @ exp(leaky(E)), and
    1/D = exp(-ln D) on the ACT engine (no slow DVE reciprocal).
  - the 3-layer output MLP runs as block-diagonal matmuls over all 5
    attention rows at once.
  - GRU gates r,z computed by ONE K=128 matmul per subtile from xh=[x;h].

Engine partition rules honored: matmul operand base partition in {0,32,64}
with lhsT base == rhs base; f32r matmul outputs only at base 0; ACT/DVE
never shift partitions (all repacking via DMA).
"""
import sys
sys.path.insert(0, '/opt/trn_rl_repo')
import numpy as np

B = 65536
NCORES = 8
BL = B // NCORES          # 8192 per core
ALPHA = 0.01

_CACHE = {}


def _g(t):  # entity t -> input-proj weight group
    return 0 if t < 5 else (1 if t < 9 else 2)


def _constants(w):
    f32 = np.float32
    win = [w['w_in0'], w['w_in1'], w['w_in2']]
    bin_ = [w['b_in0'], w['b_in1'], w['b_in2']]

    # proj: 5 matmuls, rhs = obsx rows [0:32],[0:32],[32:64],[32:64],[64:80]
    # mm j covers entity pair (2j, 2j+1): out cols 0:64 -> e=2j, 64:128 -> 2j+1
    mm_rows = [(0, 32), (0, 32), (32, 64), (32, 64), (64, 80)]
    mm_ents = [(0, 1), (2, 3), (4, 5), (6, 7), (8, 9)]
    CBD = np.zeros((80, 5 * 128), f32)
    for j, ((r0, r1), ents) in enumerate(zip(mm_rows, mm_ents)):
        for half, t in enumerate(ents):
            fr = 8 * t - r0
            CBD[r0 + fr:r0 + fr + 8, 128 * j + 64 * half:128 * j + 64 * half + 64] = win[_g(t)]
    bP = np.zeros((128, 5), f32)
    for j, ents in enumerate(mm_ents):
        bP[0:64, j] = bin_[_g(ents[0])]
        bP[64:128, j] = bin_[_g(ents[1])]

    W64 = w['W'].astype(np.float64)
    a64 = w['a'].astype(np.float64)
    Wa = np.concatenate([W64 @ a64[:64], W64 @ a64[64:]], axis=1).astype(f32)
    CL = np.zeros((128, 5 * 20), f32)       # S accumulation lhsT per pair
    for j, ents in enumerate(mm_ents):
        for half, t in enumerate(ents):
            CL[64 * half:64 * half + 64, 20 * j + 2 * t] = Wa[:, 0]
            CL[64 * half:64 * half + 64, 20 * j + 2 * t + 1] = Wa[:, 1]

    CS = np.zeros((20, 50), f32)            # E = CS.T @ S
    for i in range(5):
        for j in range(5):
            CS[2 * i, 5 * i + j] += 1            # e0(i,j) = s1[i] + s2[5+j]
            CS[2 * (5 + j) + 1, 5 * i + j] += 1
            CS[2 * (5 + j), 25 + 5 * i + j] += 1  # e1(i,j) = s1[5+j] + s2[i]
            CS[2 * i + 1, 25 + 5 * i + j] += 1

    CG = np.zeros((50, 10), f32)            # D = CG.T @ X
    for i in range(5):
        for j in range(5):
            CG[5 * i + j, i] += 1
            CG[25 + 5 * i + j, 5 + j] += 1

    Bm = np.zeros((10, 50), f32)            # Rexp = Bm.T @ R
    for i in range(5):
        for j in range(5):
            Bm[i, 5 * i + j] = 1
            Bm[5 + j, 25 + 5 * i + j] = 1
    CB2 = np.zeros((42, 50), f32)           # duplicated at rows 0 / 32
    CB2[0:10] = Bm
    CB2[32:42] = Bm

    W1 = np.zeros((50, 160), f32)           # o-MLP layer1 block diag
    for i in range(5):
        for j in range(5):
            W1[5 * i + j, 32 * i:32 * i + 32] = w['w_o1'][j]
            W1[25 + 5 * i + j, 32 * i:32 * i + 32] = w['w_o1'][5 + j]
    CW1 = W1
    b1cat = np.tile(w['b_o1'], 5)
    b1a = b1cat[0:80].reshape(-1, 1).astype(f32)
    b1b = b1cat[80:160].reshape(-1, 1).astype(f32)

    W2 = np.zeros((160, 80), f32)
    for i in range(5):
        W2[32 * i:32 * i + 32, 16 * i:16 * i + 16] = w['w_o2']
    CW2a = W2[0:80].copy()
    CW2b = W2[80:160].copy()
    b2 = np.tile(w['b_o2'], 5).reshape(-1, 1).astype(f32)

    CW3 = np.zeros((80, 5), f32)
    for i in range(5):
        CW3[16 * i:16 * i + 16, i] = w['w_o3'][:, 0]
    b3 = np.full((69, 1), float(w['b_o3'][0]), f32)

    CON5 = np.zeros((69, 1), f32)
    for r in (0, 32, 64):
        CON5[r:r + 5] = 1.0
    CON1 = np.zeros((65, 5), f32)
    for r in (0, 32, 64):
        CON1[r] = 1.0

    perm = [(4 + r) % 80 for r in range(80)] + [80, 81, 82, 83, 84]
    CF1 = np.zeros((101, 64), f32)
    CF1[0:85] = w['w_fc1'][perm]
    CF1[96:101] = w['w_fc1'][85:90]
    bf1 = w['b_fc1'].reshape(-1, 1).astype(f32)

    wihT = w['w_ih'].T.astype(f32)          # [64, 192]
    whhT = w['w_hh'].T.astype(f32)
    CRZ = np.zeros((128, 128), f32)         # [r|z] from xh=[x;h]
    CRZ[0:64, 0:64] = wihT[:, 0:64]
    CRZ[0:64, 64:128] = wihT[:, 64:128]
    CRZ[64:128, 0:64] = whhT[:, 0:64]
    CRZ[64:128, 64:128] = whhT[:, 64:128]
    brz = np.concatenate([w['b_ih'][0:64] + w['b_hh'][0:64],
                          w['b_ih'][64:128] + w['b_hh'][64:128]]
                         ).reshape(-1, 1).astype(f32)
    CNI = wihT[:, 128:192].copy()           # gin lhsT [64,64], rhs xh[0:64]
    CNH = np.zeros((128, 64), f32)          # ghn lhsT at base 64
    CNH[64:128] = whhT[:, 128:192]
    bihn = w['b_ih'][128:192].reshape(-1, 1).astype(f32)
    bhhn = w['b_hh'][128:192].reshape(-1, 1).astype(f32)

    CF2 = w['w_fc2'].astype(f32)            # [64, 5]
    bq = w['b_fc2'].reshape(-1, 1).astype(f32)

    return dict(CBD=CBD, bP=bP, CL=CL, CS=CS, CG=CG, CB2=CB2, CW1=CW1,
                b1a=b1a, b1b=b1b, CW2a=CW2a, CW2b=CW2b, b2=b2, CW3=CW3, b3=b3,
                CON5=CON5, CON1=CON1, CF1=CF1, bf1=bf1,
                CRZ=CRZ, brz=brz, CNI=CNI, CNH=CNH, bihn=bihn, bhhn=bhhn,
                CF2=CF2, bq=bq)


def _build_program(cshapes):
    import concourse.bacc as bacc
    import concourse.mybir as mybir
    from concourse.tile import TileContext

    F32 = mybir.dt.float32
    F32R = mybir.dt.float32r
    AF = mybir.ActivationFunctionType
    OP = mybir.AluOpType

    # Restrict bacc's activation-table choices: natural_log_exp_and_others
    # covers all attention-phase funcs (prelu/exp/ln/relu/copy) and
    # sigmoid_and_others covers the GRU phase (sigmoid/tanh). Emptying the
    # other sets stops the chooser from thrashing exp_and_others <->
    # natural_log (~14 x 1.3us of ACT_TABLE_LOAD otherwise).
    if not hasattr(bacc, '_orig_gat'):
        bacc._orig_gat = bacc.get_activation_tables
        _keep = {'natural_log_exp_and_others', 'sigmoid_and_others'}
        def _filtered(arch):
            t = bacc._orig_gat(arch)
            return {k: (v if k in _keep else set()) for k, v in t.items()}
        bacc.get_activation_tables = _filtered

    nc = bacc.Bacc(None, target_bir_lowering=False)

    obsx = nc.declare_dram_parameter("obsx", [96, BL], F32R, isOutput=False)
    hTD = nc.declare_dram_parameter("hTD", [64, BL], F32R, isOutput=False)
    qT = nc.declare_dram_parameter("qT", [5, BL], F32, isOutput=True)
    hoT = nc.declare_dram_parameter("hoT", [64, BL], F32R, isOutput=True)
    # all constants ship as two packed arrays (one DMA each)
    pkr = nc.declare_dram_parameter("pkr", list(cshapes['pkr']), F32R,
                                    isOutput=False)
    pkb = nc.declare_dram_parameter("pkb", list(cshapes['pkb']), F32,
                                    isOutput=False)

    mm_rows = [(0, 32), (0, 32), (32, 64), (32, 64), (64, 80)]
    slot = [(0, 0), (32, 0), (64, 0), (0, 1024)]

    with TileContext(nc) as tc:
        with tc.tile_pool(name="const", bufs=1) as cp, \
             tc.tile_pool(name="xh", bufs=8) as xhp:

            PKR = cp.tile(list(cshapes['pkr']), F32R, tag="pkr", name="PKR")
            nc.sync.dma_start(out=PKR[:], in_=pkr[:])
            PKB = cp.tile(list(cshapes['pkb']), F32, tag="pkb", name="PKB")
            nc.sync.dma_start(out=PKB[:], in_=pkb[:])
            C = {}
            for name, (p0, p1, c0, c1) in cshapes['rmap'].items():
                C[name] = PKR[p0:p1, c0:c1]
            for name, (p0, p1, c0, c1) in cshapes['bmap'].items():
                C[name] = PKB[p0:p1, c0:c1]

            XH = []

            with tc.tile_pool(name="og", bufs=5) as ogp, \
                 tc.tile_pool(name="hp", bufs=3) as hpp, \
                 tc.tile_pool(name="sc", bufs=2) as scp, \
                 tc.tile_pool(name="xf", bufs=2) as xfp, \
                 tc.tile_pool(name="df", bufs=2) as dfp, \
                 tc.tile_pool(name="h1", bufs=2) as h1p, \
                 tc.tile_pool(name="h2", bufs=2) as h2p, \
                 tc.tile_pool(name="h3", bufs=4) as h3p, \
                 tc.tile_pool(name="pp", bufs=2, space="PSUM") as ppp, \
                 tc.tile_pool(name="ch", bufs=2, space="PSUM") as chp:
                for xt in range(2):          # 2 blocks x 4 pairs x 1024 cols
                    OGs = []
                    XEs = []
                    DPs = [None, None]
                    RPs = [None, None]
                    for uu in range(4):
                        u = 4 * xt + uu
                        OG = ogp.tile([101, 1024], F32R, tag="OG", name="OG")
                        nc.sync.dma_start(out=OG[0:96, :],
                                          in_=obsx[:, 1024 * u:1024 * (u + 1)])
                        OGs.append(OG)
                        S2 = chp.tile([20, 1024], F32, tag="ch", name="S2")
                        for j in range(5):
                            r0, r1 = mm_rows[j]
                            PP = ppp.tile([128, 1024], F32, tag="pp", name="PP")
                            for h in range(2):
                                nc.tensor.matmul(
                                    PP[:, 512 * h:512 * (h + 1)],
                                    C['CBD'][r0:r1, 128 * j:128 * (j + 1)],
                                    OG[r0:r1, 512 * h:512 * (h + 1)],
                                    start=True, stop=True)
                            Hp = hpp.tile([128, 1024], F32R, tag="Hp",
                                          name="Hp")
                            nc.scalar.activation(Hp[:], PP[:], AF.Prelu,
                                                 bias=C['bP'][:, j:j + 1],
                                                 alpha=ALPHA)
                            for h in range(2):
                                nc.tensor.matmul(
                                    S2[:, 512 * h:512 * (h + 1)],
                                    C['CL'][:, 20 * j:20 * (j + 1)],
                                    Hp[:, 512 * h:512 * (h + 1)],
                                    start=(j == 0), stop=(j == 4))
                        SC = scp.tile([20, 1024], F32R, tag="SC", name="SC")
                        nc.vector.tensor_copy(SC[:], S2[:])
                        E2 = chp.tile([50, 1024], F32, tag="ch", name="E2")
                        for h in range(2):
                            nc.tensor.matmul(E2[:, 512 * h:512 * (h + 1)],
                                             C['CS'][:],
                                             SC[:, 512 * h:512 * (h + 1)],
                                             start=True, stop=True)
                        XP = xfp.tile([50, 1024], F32, tag="XP", name="XP")
                        nc.scalar.activation(XP[:], E2[:], AF.Prelu,
                                             alpha=ALPHA)
                        XE = xfp.tile([50, 1024], F32R, tag="XE", bufs=4,
                                      name="XE")
                        nc.scalar.activation(XE[:], XP[:], AF.Exp)
                        XEs.append(XE)
                        D2 = chp.tile([10, 1024], F32, tag="ch", name="D2")
                        for h in range(2):
                            nc.tensor.matmul(D2[:, 512 * h:512 * (h + 1)],
                                             C['CG'][:],
                                             XE[:, 512 * h:512 * (h + 1)],
                                             start=True, stop=True)
                        du, k = uu // 2, uu % 2
                        if k == 0:
                            DPs[du] = dfp.tile([42, 1024], F32, tag="DP",
                                               name="DPt")
                        nc.vector.tensor_copy(DPs[du][32 * k:32 * k + 10, :],
                                              D2[:])
                        if k == 1:
                            LD = dfp.tile([42, 1024], F32, tag="LD", bufs=1,
                                          name="LD")
                            nc.scalar.activation(LD[:], DPs[du][:], AF.Ln)
                            RP = dfp.tile([42, 1024], F32R, tag="RP", name="RP")
                            nc.scalar.activation(RP[:], LD[:], AF.Exp,
                                                 scale=-1.0)
                            RPs[du] = RP

                    # attention = X * (1/D broadcast), in place over XE
                    for uu in range(4):
                        rb2 = 32 * (uu % 2)
                        RP = RPs[uu // 2]
                        RE = chp.tile([50, 1024], F32, tag="ch", name="RE")
                        for h in range(2):
                            nc.tensor.matmul(
                                RE[:, 512 * h:512 * (h + 1)],
                                C['CB2'][rb2:rb2 + 10, :],
                                RP[rb2:rb2 + 10, 512 * h:512 * (h + 1)],
                                start=True, stop=True)
                        nc.vector.tensor_tensor(XEs[uu][:],
                                                XEs[uu][:].bitcast(F32),
                                                RE[:], OP.mult)

                    # o-MLP (3 leaky layers, block diagonal over the 5 rows)
                    H3P = h3p.tile([69, 2048], F32, tag="h3", name="H3P")
                    for uu in range(4):
                        att = XEs[uu]
                        P1a = chp.tile([80, 1024], F32, tag="ch", name="P1a")
                        P1b = chp.tile([80, 1024], F32, tag="ch", name="P1b")
                        for h in range(2):
                            nc.tensor.matmul(P1a[:, 512 * h:512 * (h + 1)],
                                             C['CW1'][:, 0:80],
                                             att[:, 512 * h:512 * (h + 1)],
                                             start=True, stop=True)
                            nc.tensor.matmul(P1b[:, 512 * h:512 * (h + 1)],
                                             C['CW1'][:, 80:160],
                                             att[:, 512 * h:512 * (h + 1)],
                                             start=True, stop=True)
                        H1A = h1p.tile([80, 1024], F32R, tag="H1A", name="H1A")
                        H1B = h1p.tile([80, 1024], F32R, tag="H1B", name="H1B")
                        nc.scalar.activation(H1A[:], P1a[:], AF.Prelu,
                                             bias=C['b1a'][:], alpha=ALPHA)
                        nc.scalar.activation(H1B[:], P1b[:], AF.Prelu,
                                             bias=C['b1b'][:], alpha=ALPHA)
                        P2 = chp.tile([80, 1024], F32, tag="ch", name="P2")
                        for h in range(2):
                            nc.tensor.matmul(P2[:, 512 * h:512 * (h + 1)],
                                             C['CW2a'][:],
                                             H1A[:, 512 * h:512 * (h + 1)],
                                             start=True, stop=False)
                            nc.tensor.matmul(P2[:, 512 * h:512 * (h + 1)],
                                             C['CW2b'][:],
                                             H1B[:, 512 * h:512 * (h + 1)],
                                             start=False, stop=True)
                        H2 = h2p.tile([80, 1024], F32R, tag="H2", name="H2")
                        nc.scalar.activation(H2[:], P2[:], AF.Prelu,
                                             bias=C['b2'][:], alpha=ALPHA)
                        P3 = chp.tile([5, 1024], F32, tag="ch", name="P3")
                        for h in range(2):
                            nc.tensor.matmul(P3[:, 512 * h:512 * (h + 1)],
                                             C['CW3'][:],
                                             H2[:, 512 * h:512 * (h + 1)],
                                             start=True, stop=True)
                        r3, f3 = slot[uu]
                        nc.vector.tensor_copy(H3P[r3:r3 + 5, f3:f3 + 1024],
                                              P3[:])
                    X3P = h3p.tile([69, 2048], F32R, tag="h3", name="X3P")
                    nc.scalar.activation(X3P[:], H3P[:], AF.Prelu,
                                         bias=C['b3'][:], alpha=ALPHA)
                    nc.scalar.activation(X3P[:], X3P[:].bitcast(F32), AF.Exp)

                    D3P = h3p.tile([65, 2048], F32, tag="h3", name="D3P")
                    for uu in range(4):
                        r3, f3 = slot[uu]
                        PD3 = chp.tile([1, 1024], F32, tag="ch", name="PD3")
                        for h in range(2):
                            nc.tensor.matmul(
                                PD3[:, 512 * h:512 * (h + 1)],
                                C['CON5'][r3:r3 + 5, :],
                                X3P[r3:r3 + 5, f3 + 512 * h:f3 + 512 * (h + 1)],
                                start=True, stop=True)
                        nc.vector.tensor_copy(D3P[r3:r3 + 1, f3:f3 + 1024],
                                              PD3[:])
                    L3 = h3p.tile([65, 2048], F32, tag="h3", name="L3")
                    nc.scalar.activation(L3[:], D3P[:], AF.Ln)
                    R3P = h3p.tile([65, 2048], F32R, tag="h3", name="R3P")
                    nc.scalar.activation(R3P[:], L3[:], AF.Exp, scale=-1.0)

                    # obs_out = X3 * (1/D3) -> OG rows 96:101, then fc1 K=101
                    for uu in range(4):
                        u = 4 * xt + uu
                        r3, f3 = slot[uu]
                        R5 = chp.tile([5, 1024], F32, tag="ch", name="R5")
                        for h in range(2):
                            nc.tensor.matmul(
                                R5[:, 512 * h:512 * (h + 1)],
                                C['CON1'][r3:r3 + 1, :],
                                R3P[r3:r3 + 1, f3 + 512 * h:f3 + 512 * (h + 1)],
                                start=True, stop=True)
                        nc.vector.tensor_tensor(
                            OGs[uu][96:101, :],
                            X3P[r3:r3 + 5, f3:f3 + 1024].bitcast(F32),
                            R5[:], OP.mult)
                        PX = chp.tile([64, 1024], F32, tag="ch", name="PX")
                        for h in range(2):
                            nc.tensor.matmul(PX[:, 512 * h:512 * (h + 1)],
                                             C['CF1'][:],
                                             OGs[uu][:, 512 * h:512 * (h + 1)],
                                             start=True, stop=True)
                        xh = xhp.tile([128, 1024], F32R, tag="xh", name="xh")
                        nc.scalar.activation(xh[0:64, :], PX[:], AF.Relu,
                                             bias=C['bf1'][:])
                        nc.gpsimd.dma_start(
                            out=xh[64:128, :],
                            in_=hTD[:, 1024 * u:1024 * (u + 1)])
                        XH.append(xh)

            # ----- PASS B: GRU + fc2 (sigmoid/tanh activation table) -------
            with tc.tile_pool(name="gb", bufs=2) as gbp, \
                 tc.tile_pool(name="vt", bufs=3) as vtp, \
                 tc.tile_pool(name="ho", bufs=2) as hop, \
                 tc.tile_pool(name="gps", bufs=3, space="PSUM") as gps, \
                 tc.tile_pool(name="qps", bufs=1, space="PSUM") as qps:
                for u in range(8):
                    xh = XH[u]
                    PRZ = gps.tile([128, 1024], F32, tag="g", name="PRZ")
                    for h in range(2):
                        nc.tensor.matmul(PRZ[:, 512 * h:512 * (h + 1)],
                                         C['CRZ'][:],
                                         xh[:, 512 * h:512 * (h + 1)],
                                         start=True, stop=True)
                    RZS = gbp.tile([128, 1024], F32, tag="RZS", name="RZS")
                    nc.scalar.activation(RZS[:], PRZ[:], AF.Sigmoid,
                                         bias=C['brz'][:])
                    Z64 = gbp.tile([64, 1024], F32, tag="Z64", bufs=1,
                                   name="Z64")
                    nc.gpsimd.dma_start(out=Z64[:], in_=RZS[64:128, :])
                    PGI = gps.tile([64, 1024], F32, tag="g", name="PGI")
                    PGH = gps.tile([64, 1024], F32, tag="g", name="PGH")
                    for h in range(2):
                        nc.tensor.matmul(PGI[:, 512 * h:512 * (h + 1)],
                                         C['CNI'][:],
                                         xh[0:64, 512 * h:512 * (h + 1)],
                                         start=True, stop=True)
                        nc.tensor.matmul(PGH[:, 512 * h:512 * (h + 1)],
                                         C['CNH'][64:128, :],
                                         xh[64:128, 512 * h:512 * (h + 1)],
                                         start=True, stop=True)
                    V1 = vtp.tile([64, 1024], F32, tag="vt", name="V1")
                    nc.vector.scalar_tensor_tensor(V1[:], PGH[:],
                                                   C['bhhn'][:],
                                                   RZS[0:64, :],
                                                   OP.add, OP.mult)
                    V3 = vtp.tile([64, 1024], F32, tag="vt", name="V3")
                    nc.vector.tensor_tensor(V3[:], V1[:], PGI[:], OP.add)
                    N2 = gbp.tile([64, 1024], F32, tag="N2", name="N2")
                    nc.scalar.activation(N2[:], V3[:], AF.Tanh,
                                         bias=C['bihn'][:])
                    H64 = gbp.tile([64, 1024], F32R, tag="H64", bufs=2,
                                   name="H64")
                    nc.gpsimd.dma_start(out=H64[:],
                                        in_=hTD[:, 1024 * u:1024 * (u + 1)])
                    V4 = vtp.tile([64, 1024], F32, tag="vt", name="V4")
                    nc.vector.tensor_tensor(V4[:], H64[:].bitcast(F32),
                                            N2[:], OP.subtract)
                    V5 = vtp.tile([64, 1024], F32, tag="vt", name="V5")
                    nc.vector.tensor_tensor(V5[:], Z64[:], V4[:], OP.mult)
                    HO = hop.tile([64, 1024], F32R, tag="HO", name="HO")
                    nc.vector.tensor_tensor(HO[:], N2[:], V5[:], OP.add)
                    nc.gpsimd.dma_start(out=hoT[:, 1024 * u:1024 * (u + 1)],
                                        in_=HO[:])
                    Q2 = qps.tile([5, 1024], F32, tag="q", name="Q2")
                    for h in range(2):
                        nc.tensor.matmul(Q2[:, 512 * h:512 * (h + 1)],
                                         C['CF2'][:],
                                         HO[:, 512 * h:512 * (h + 1)],
                                         start=True, stop=True)
                    QS = gbp.tile([5, 1024], F32, tag="QS", bufs=1, name="QS")
                    nc.vector.tensor_scalar_add(QS[:], Q2[:], C['bq'][:])
                    nc.gpsimd.dma_start(out=qT[:, 1024 * u:1024 * (u + 1)],
                                        in_=QS[:])

    nc.compile()
    return nc


def _pack_consts(consts):
    """Pack all constant arrays into two [128, N] arrays (f32r-bound ones and
    fp32 biases) so startup is 2 DMAs instead of 26. Returns (pkr, pkb,
    shapes-dict with slice maps)."""
    BIASN = {'bP', 'b1a', 'b1b', 'b2', 'b3', 'bf1', 'brz', 'bihn', 'bhhn',
             'bq'}
    rmap, bmap = {}, {}
    rcols = bcols = 0
    for name, a in consts.items():
        if name in BIASN:
            bmap[name] = (0, a.shape[0], bcols, bcols + a.shape[1])
            bcols += a.shape[1]
        else:
            rmap[name] = (0, a.shape[0], rcols, rcols + a.shape[1])
            rcols += a.shape[1]
    pkr = np.zeros((128, rcols), np.float32)
    pkb = np.zeros((128, bcols), np.float32)
    for name, a in consts.items():
        if name in BIASN:
            p0, p1, c0, c1 = bmap[name]
            pkb[p0:p1, c0:c1] = a
        else:
            p0, p1, c0, c1 = rmap[name]
            pkr[p0:p1, c0:c1] = a
    return pkr, pkb, rmap, bmap


def kernel(**inputs):
    from concourse.bass_utils import run_bass_kernel_spmd

    obs = np.asarray(inputs['obs'], np.float32)
    hid = np.asarray(inputs['hidden_state'], np.float32)
    consts = _constants({k: np.asarray(v, np.float32)
                         for k, v in inputs.items()
                         if k not in ('obs', 'hidden_state')})
    pkr, pkb, rmap, bmap = _pack_consts(consts)
    cshapes = {'pkr': pkr.shape, 'pkb': pkb.shape, 'rmap': rmap, 'bmap': bmap}

    if 'nc' not in _CACHE:
        _CACHE['nc'] = _build_program(cshapes)
    nc = _CACHE['nc']

    perm = [(4 + r) % 80 for r in range(80)] + [80, 81, 82, 83, 84]
    obsT = np.zeros((96, B), np.float32)                  # rows 85:96 stay 0
    obsT[0:85] = obs.T[perm]
    hT = np.ascontiguousarray(hid.T)                      # [64, B]

    in_maps = []
    for c in range(NCORES):
        cols = slice(c * BL, (c + 1) * BL)
        m = {'obsx': np.ascontiguousarray(obsT[:, cols]),
             'hTD': np.ascontiguousarray(hT[:, cols]),
             'pkr': pkr, 'pkb': pkb}
        in_maps.append(m)

    res = run_bass_kernel_spmd(nc, in_maps, list(range(NCORES)))

    q = np.empty((B, 5), np.float32)
    h = np.empty((B, 64), np.float32)
    for c, r in enumerate(res.results):
        cols = slice(c * BL, (c + 1) * BL)
        q[cols] = r['qT'].T
        h[cols] = r['hoT'].T
    return q, h


# revision 25
# speedup vs baseline: 1.3882x; 1.2392x over previous
"""Trainium2 Bass kernel for nn_ATOM_RNN (GAT-entity attention + GRU cell).

Data-parallel over 8 NeuronCores: batch 65536 -> 8192/core.
Feature-on-partition layout: activations stored [feat, batch]; stored [in,out]
weights are used directly as matmul lhsT. float32r (rounded fp32, 1 cyc/row
on PE at N>=256) for every matmul operand.

Key restructurings vs the reference graph:
  - Wh @ a[:64] == h_mix @ (W @ a[:64]): the [64,64] GAT matmul is folded
    into two 64->1 projections (host precomputes W@a).
  - e/softmax over the two 5x5 blocks is built with selection matmuls
    (E = Ssel.T @ S), denominators D = Gsum.T @ exp(leaky(E)), and
    1/D = exp(-ln D) on the ACT engine (no slow DVE reciprocal).
  - the 3-layer output MLP runs as block-diagonal matmuls over all 5
    attention rows at once.
  - GRU gates r,z computed by ONE K=128 matmul per subtile from xh=[x;h].

Engine partition rules honored: matmul operand base partition in {0,32,64}
with lhsT base == rhs base; f32r matmul outputs only at base 0; ACT/DVE
never shift partitions (all repacking via DMA).
"""
import sys
sys.path.insert(0, '/opt/trn_rl_repo')
import numpy as np

B = 65536
NCORES = 8
BL = B // NCORES          # 8192 per core
ALPHA = 0.01

_CACHE = {}


def _g(t):  # entity t -> input-proj weight group
    return 0 if t < 5 else (1 if t < 9 else 2)


def _constants(w):
    f32 = np.float32
    win = [w['w_in0'], w['w_in1'], w['w_in2']]
    bin_ = [w['b_in0'], w['b_in1'], w['b_in2']]

    # proj: 5 matmuls, rhs = obsx rows [0:32],[0:32],[32:64],[32:64],[64:80]
    # mm j covers entity pair (2j, 2j+1): out cols 0:64 -> e=2j, 64:128 -> 2j+1
    mm_rows = [(0, 32), (0, 32), (32, 64), (32, 64), (64, 80)]
    mm_ents = [(0, 1), (2, 3), (4, 5), (6, 7), (8, 9)]
    CBD = np.zeros((80, 5 * 128), f32)
    for j, ((r0, r1), ents) in enumerate(zip(mm_rows, mm_ents)):
        for half, t in enumerate(ents):
            fr = 8 * t - r0
            CBD[r0 + fr:r0 + fr + 8, 128 * j + 64 * half:128 * j + 64 * half + 64] = win[_g(t)]
    bP = np.zeros((128, 5), f32)
    for j, ents in enumerate(mm_ents):
        bP[0:64, j] = bin_[_g(ents[0])]
        bP[64:128, j] = bin_[_g(ents[1])]

    W64 = w['W'].astype(np.float64)
    a64 = w['a'].astype(np.float64)
    Wa = np.concatenate([W64 @ a64[:64], W64 @ a64[64:]], axis=1).astype(f32)
    CL = np.zeros((128, 5 * 20), f32)       # S accumulation lhsT per pair
    for j, ents in enumerate(mm_ents):
        for half, t in enumerate(ents):
            CL[64 * half:64 * half + 64, 20 * j + 2 * t] = Wa[:, 0]
            CL[64 * half:64 * half + 64, 20 * j + 2 * t + 1] = Wa[:, 1]

    CS = np.zeros((20, 50), f32)            # E = CS.T @ S
    for i in range(5):
        for j in range(5):
            CS[2 * i, 5 * i + j] += 1            # e0(i,j) = s1[i] + s2[5+j]
            CS[2 * (5 + j) + 1, 5 * i + j] += 1
            CS[2 * (5 + j), 25 + 5 * i + j] += 1  # e1(i,j) = s1[5+j] + s2[i]
            CS[2 * i + 1, 25 + 5 * i + j] += 1

    CG = np.zeros((50, 10), f32)            # D = CG.T @ X
    for i in range(5):
        for j in range(5):
            CG[5 * i + j, i] += 1
            CG[25 + 5 * i + j, 5 + j] += 1

    Bm = np.zeros((10, 50), f32)            # Rexp = Bm.T @ R
    for i in range(5):
        for j in range(5):
            Bm[i, 5 * i + j] = 1
            Bm[5 + j, 25 + 5 * i + j] = 1
    CB2 = np.zeros((42, 50), f32)           # duplicated at rows 0 / 32
    CB2[0:10] = Bm
    CB2[32:42] = Bm

    W1 = np.zeros((50, 160), f32)           # o-MLP layer1 block diag
    for i in range(5):
        for j in range(5):
            W1[5 * i + j, 32 * i:32 * i + 32] = w['w_o1'][j]
            W1[25 + 5 * i + j, 32 * i:32 * i + 32] = w['w_o1'][5 + j]
    CW1 = W1
    b1cat = np.tile(w['b_o1'], 5)
    b1a = b1cat[0:80].reshape(-1, 1).astype(f32)
    b1b = b1cat[80:160].reshape(-1, 1).astype(f32)

    W2 = np.zeros((160, 80), f32)
    for i in range(5):
        W2[32 * i:32 * i + 32, 16 * i:16 * i + 16] = w['w_o2']
    CW2a = W2[0:80].copy()
    CW2b = W2[80:160].copy()
    b2 = np.tile(w['b_o2'], 5).reshape(-1, 1).astype(f32)

    CW3 = np.zeros((80, 5), f32)
    for i in range(5):
        CW3[16 * i:16 * i + 16, i] = w['w_o3'][:, 0]
    b3 = np.full((69, 1), float(w['b_o3'][0]), f32)

    CON5 = np.zeros((69, 1), f32)
    for r in (0, 32, 64):
        CON5[r:r + 5] = 1.0
    CON1 = np.zeros((65, 5), f32)
    for r in (0, 32, 64):
        CON1[r] = 1.0

    perm = [(4 + r) % 80 for r in range(80)] + [80, 81, 82, 83, 84]
    CF1 = np.zeros((101, 64), f32)
    CF1[0:85] = w['w_fc1'][perm]
    CF1[96:101] = w['w_fc1'][85:90]
    bf1 = w['b_fc1'].reshape(-1, 1).astype(f32)

    wihT = w['w_ih'].T.astype(f32)          # [64, 192]
    whhT = w['w_hh'].T.astype(f32)
    CRZ = np.zeros((128, 128), f32)         # [r|z] from xh=[x;h]
    CRZ[0:64, 0:64] = wihT[:, 0:64]
    CRZ[0:64, 64:128] = wihT[:, 64:128]
    CRZ[64:128, 0:64] = whhT[:, 0:64]
    CRZ[64:128, 64:128] = whhT[:, 64:128]
    brz = np.concatenate([w['b_ih'][0:64] + w['b_hh'][0:64],
                          w['b_ih'][64:128] + w['b_hh'][64:128]]
                         ).reshape(-1, 1).astype(f32)
    CNI = wihT[:, 128:192].copy()           # gin lhsT [64,64], rhs xh[0:64]
    CNH = np.zeros((128, 64), f32)          # ghn lhsT at base 64
    CNH[64:128] = whhT[:, 128:192]
    bihn = w['b_ih'][128:192].reshape(-1, 1).astype(f32)
    bhhn = w['b_hh'][128:192].reshape(-1, 1).astype(f32)

    CF2 = np.concatenate([w['w_fc2'], w['w_fc2']], axis=0).astype(f32)  # [128,5]
    bq = w['b_fc2'].reshape(-1, 1).astype(f32)

    return dict(CBD=CBD, bP=bP, CL=CL, CS=CS, CG=CG, CB2=CB2, CW1=CW1,
                b1a=b1a, b1b=b1b, CW2a=CW2a, CW2b=CW2b, b2=b2, CW3=CW3, b3=b3,
                CON5=CON5, CON1=CON1, CF1=CF1, bf1=bf1,
                CRZ=CRZ, brz=brz, CNI=CNI, CNH=CNH, bihn=bihn, bhhn=bhhn,
                CF2=CF2, bq=bq)


def _build_program(cshapes):
    import concourse.bacc as bacc
    import concourse.mybir as mybir
    from concourse.tile import TileContext

    F32 = mybir.dt.float32
    F32R = mybir.dt.float32r
    AF = mybir.ActivationFunctionType
    OP = mybir.AluOpType

    # Restrict bacc's activation-table choices: natural_log_exp_and_others
    # covers all attention-phase funcs (prelu/exp/ln/relu/copy) and
    # sigmoid_and_others covers the GRU phase (sigmoid/tanh). Emptying the
    # other sets stops the chooser from thrashing exp_and_others <->
    # natural_log (~14 x 1.3us of ACT_TABLE_LOAD otherwise).
    if not hasattr(bacc, '_orig_gat'):
        bacc._orig_gat = bacc.get_activation_tables
        _keep = {'natural_log_exp_and_others', 'sigmoid_and_others'}
        def _filtered(arch):
            t = bacc._orig_gat(arch)
            return {k: (v if k in _keep else set()) for k, v in t.items()}
        bacc.get_activation_tables = _filtered

    nc = bacc.Bacc(None, target_bir_lowering=False)

    obsx = nc.declare_dram_parameter("obsx", [96, BL], F32R, isOutput=False)
    hTD = nc.declare_dram_parameter("hTD", [64, BL], F32R, isOutput=False)
    qT = nc.declare_dram_parameter("qT", [5, BL], F32, isOutput=True)
    hoT = nc.declare_dram_parameter("hoT", [64, BL], F32R, isOutput=True)
    # all constants ship as two packed arrays (one DMA each)
    pkr = nc.declare_dram_parameter("pkr", list(cshapes['pkr']), F32R,
                                    isOutput=False)
    pkb = nc.declare_dram_parameter("pkb", list(cshapes['pkb']), F32,
                                    isOutput=False)

    mm_rows = [(0, 32), (0, 32), (32, 64), (32, 64), (64, 80)]
    slot = [(0, 0), (32, 0), (64, 0), (0, 1024)]

    with TileContext(nc) as tc:
        with tc.tile_pool(name="const", bufs=1) as cp, \
             tc.tile_pool(name="xh", bufs=8) as xhp:

            PKR = cp.tile(list(cshapes['pkr']), F32R, tag="pkr", name="PKR")
            nc.sync.dma_start(out=PKR[:], in_=pkr[:])
            PKB = cp.tile(list(cshapes['pkb']), F32, tag="pkb", name="PKB")
            nc.sync.dma_start(out=PKB[:], in_=pkb[:])
            C = {}
            for name, (p0, p1, c0, c1) in cshapes['rmap'].items():
                C[name] = PKR[p0:p1, c0:c1]
            for name, (p0, p1, c0, c1) in cshapes['bmap'].items():
                C[name] = PKB[p0:p1, c0:c1]

            XH = []

            with tc.tile_pool(name="og", bufs=5) as ogp, \
                 tc.tile_pool(name="hp", bufs=3) as hpp, \
                 tc.tile_pool(name="sc", bufs=2) as scp, \
                 tc.tile_pool(name="xf", bufs=2) as xfp, \
                 tc.tile_pool(name="df", bufs=2) as dfp, \
                 tc.tile_pool(name="h1", bufs=2) as h1p, \
                 tc.tile_pool(name="h2", bufs=2) as h2p, \
                 tc.tile_pool(name="h3", bufs=4) as h3p, \
                 tc.tile_pool(name="pp", bufs=2, space="PSUM") as ppp, \
                 tc.tile_pool(name="ch", bufs=2, space="PSUM") as chp:
                for xt in range(2):          # 2 blocks x 4 pairs x 1024 cols
                    OGs = []
                    XEs = []
                    DPs = [None, None]
                    RPs = [None, None]
                    for uu in range(4):
                        u = 4 * xt + uu
                        OG = ogp.tile([101, 1024], F32R, tag="OG", name="OG")
                        nc.sync.dma_start(out=OG[0:96, :],
                                          in_=obsx[:, 1024 * u:1024 * (u + 1)])
                        OGs.append(OG)
                        S2 = chp.tile([20, 1024], F32, tag="ch", name="S2")
                        for j in range(5):
                            r0, r1 = mm_rows[j]
                            PP = ppp.tile([128, 1024], F32, tag="pp", name="PP")
                            for h in range(2):
                                nc.tensor.matmul(
                                    PP[:, 512 * h:512 * (h + 1)],
                                    C['CBD'][r0:r1, 128 * j:128 * (j + 1)],
                                    OG[r0:r1, 512 * h:512 * (h + 1)],
                                    start=True, stop=True)
                            Hp = hpp.tile([128, 1024], F32R, tag="Hp",
                                          name="Hp")
                            nc.scalar.activation(Hp[:], PP[:], AF.Prelu,
                                                 bias=C['bP'][:, j:j + 1],
                                                 alpha=ALPHA)
                            for h in range(2):
                                nc.tensor.matmul(
                                    S2[:, 512 * h:512 * (h + 1)],
                                    C['CL'][:, 20 * j:20 * (j + 1)],
                                    Hp[:, 512 * h:512 * (h + 1)],
                                    start=(j == 0), stop=(j == 4))
                        SC = scp.tile([20, 1024], F32R, tag="SC", name="SC")
                        nc.vector.tensor_copy(SC[:], S2[:])
                        E2 = chp.tile([50, 1024], F32, tag="ch", name="E2")
                        for h in range(2):
                            nc.tensor.matmul(E2[:, 512 * h:512 * (h + 1)],
                                             C['CS'][:],
                                             SC[:, 512 * h:512 * (h + 1)],
                                             start=True, stop=True)
                        XP = xfp.tile([50, 1024], F32, tag="XP", name="XP")
                        nc.scalar.activation(XP[:], E2[:], AF.Prelu,
                                             alpha=ALPHA)
                        XE = xfp.tile([50, 1024], F32R, tag="XE", bufs=4,
                                      name="XE")
                        nc.scalar.activation(XE[:], XP[:], AF.Exp)
                        XEs.append(XE)
                        D2 = chp.tile([10, 1024], F32, tag="ch", name="D2")
                        for h in range(2):
                            nc.tensor.matmul(D2[:, 512 * h:512 * (h + 1)],
                                             C['CG'][:],
                                             XE[:, 512 * h:512 * (h + 1)],
                                             start=True, stop=True)
                        du, k = uu // 2, uu % 2
                        if k == 0:
                            DPs[du] = dfp.tile([42, 1024], F32, tag="DP",
                                               name="DPt")
                        nc.vector.tensor_copy(DPs[du][32 * k:32 * k + 10, :],
                                              D2[:])
                        if k == 1:
                            LD = dfp.tile([42, 1024], F32, tag="LD", bufs=1,
                                          name="LD")
                            nc.scalar.activation(LD[:], DPs[du][:], AF.Ln)
                            RP = dfp.tile([42, 1024], F32R, tag="RP", name="RP")
                            nc.scalar.activation(RP[:], LD[:], AF.Exp,
                                                 scale=-1.0)
                            RPs[du] = RP

                    # attention = X * (1/D broadcast), in place over XE
                    for uu in range(4):
                        rb2 = 32 * (uu % 2)
                        RP = RPs[uu // 2]
                        RE = chp.tile([50, 1024], F32, tag="ch", name="RE")
                        for h in range(2):
                            nc.tensor.matmul(
                                RE[:, 512 * h:512 * (h + 1)],
                                C['CB2'][rb2:rb2 + 10, :],
                                RP[rb2:rb2 + 10, 512 * h:512 * (h + 1)],
                                start=True, stop=True)
                        nc.vector.tensor_tensor(XEs[uu][:],
                                                XEs[uu][:].bitcast(F32),
                                                RE[:], OP.mult)

                    # o-MLP (3 leaky layers, block diagonal over the 5 rows)
                    H3P = h3p.tile([69, 2048], F32, tag="h3", name="H3P")
                    for uu in range(4):
                        att = XEs[uu]
                        P1a = chp.tile([80, 1024], F32, tag="ch", name="P1a")
                        P1b = chp.tile([80, 1024], F32, tag="ch", name="P1b")
                        for h in range(2):
                            nc.tensor.matmul(P1a[:, 512 * h:512 * (h + 1)],
                                             C['CW1'][:, 0:80],
                                             att[:, 512 * h:512 * (h + 1)],
                                             start=True, stop=True)
                            nc.tensor.matmul(P1b[:, 512 * h:512 * (h + 1)],
                                             C['CW1'][:, 80:160],
                                             att[:, 512 * h:512 * (h + 1)],
                                             start=True, stop=True)
                        H1A = h1p.tile([80, 1024], F32R, tag="H1A", name="H1A")
                        H1B = h1p.tile([80, 1024], F32R, tag="H1B", name="H1B")
                        nc.scalar.activation(H1A[:], P1a[:], AF.Prelu,
                                             bias=C['b1a'][:], alpha=ALPHA)
                        nc.scalar.activation(H1B[:], P1b[:], AF.Prelu,
                                             bias=C['b1b'][:], alpha=ALPHA)
                        P2 = chp.tile([80, 1024], F32, tag="ch", name="P2")
                        for h in range(2):
                            nc.tensor.matmul(P2[:, 512 * h:512 * (h + 1)],
                                             C['CW2a'][:],
                                             H1A[:, 512 * h:512 * (h + 1)],
                                             start=True, stop=False)
                            nc.tensor.matmul(P2[:, 512 * h:512 * (h + 1)],
                                             C['CW2b'][:],
                                             H1B[:, 512 * h:512 * (h + 1)],
                                             start=False, stop=True)
                        H2 = h2p.tile([80, 1024], F32R, tag="H2", name="H2")
                        nc.scalar.activation(H2[:], P2[:], AF.Prelu,
                                             bias=C['b2'][:], alpha=ALPHA)
                        P3 = chp.tile([5, 1024], F32, tag="ch", name="P3")
                        for h in range(2):
                            nc.tensor.matmul(P3[:, 512 * h:512 * (h + 1)],
                                             C['CW3'][:],
                                             H2[:, 512 * h:512 * (h + 1)],
                                             start=True, stop=True)
                        r3, f3 = slot[uu]
                        nc.vector.tensor_copy(H3P[r3:r3 + 5, f3:f3 + 1024],
                                              P3[:])
                    X3P = h3p.tile([69, 2048], F32R, tag="h3", name="X3P")
                    nc.scalar.activation(X3P[:], H3P[:], AF.Prelu,
                                         bias=C['b3'][:], alpha=ALPHA)
                    nc.scalar.activation(X3P[:], X3P[:].bitcast(F32), AF.Exp)

                    D3P = h3p.tile([65, 2048], F32, tag="h3", name="D3P")
                    for uu in range(4):
                        r3, f3 = slot[uu]
                        PD3 = chp.tile([1, 1024], F32, tag="ch", name="PD3")
                        for h in range(2):
                            nc.tensor.matmul(
                                PD3[:, 512 * h:512 * (h + 1)],
                                C['CON5'][r3:r3 + 5, :],
                                X3P[r3:r3 + 5, f3 + 512 * h:f3 + 512 * (h + 1)],
                                start=True, stop=True)
                        nc.vector.tensor_copy(D3P[r3:r3 + 1, f3:f3 + 1024],
                                              PD3[:])
                    L3 = h3p.tile([65, 2048], F32, tag="h3", name="L3")
                    nc.scalar.activation(L3[:], D3P[:], AF.Ln)
                    R3P = h3p.tile([65, 2048], F32R, tag="h3", name="R3P")
                    nc.scalar.activation(R3P[:], L3[:], AF.Exp, scale=-1.0)

                    # obs_out = X3 * (1/D3) -> OG rows 96:101, then fc1 K=101
                    for uu in range(4):
                        u = 4 * xt + uu
                        r3, f3 = slot[uu]
                        R5 = chp.tile([5, 1024], F32, tag="ch", name="R5")
                        for h in range(2):
                            nc.tensor.matmul(
                                R5[:, 512 * h:512 * (h + 1)],
                                C['CON1'][r3:r3 + 1, :],
                                R3P[r3:r3 + 1, f3 + 512 * h:f3 + 512 * (h + 1)],
                                start=True, stop=True)
                        nc.vector.tensor_tensor(
                            OGs[uu][96:101, :],
                            X3P[r3:r3 + 5, f3:f3 + 1024].bitcast(F32),
                            R5[:], OP.mult)
                        PX = chp.tile([64, 1024], F32, tag="ch", name="PX")
                        for h in range(2):
                            nc.tensor.matmul(PX[:, 512 * h:512 * (h + 1)],
                                             C['CF1'][:],
                                             OGs[uu][:, 512 * h:512 * (h + 1)],
                                             start=True, stop=True)
                        xh = xhp.tile([128, 1024], F32R, tag="xh", name="xh")
                        nc.scalar.activation(xh[0:64, :], PX[:], AF.Relu,
                                             bias=C['bf1'][:])
                        nc.gpsimd.dma_start(
                            out=xh[64:128, :],
                            in_=hTD[:, 1024 * u:1024 * (u + 1)])
                        XH.append(xh)

            # ----- PASS B: GRU + fc2 (sigmoid/tanh activation table) -------
            with tc.tile_pool(name="gb", bufs=2) as gbp, \
                 tc.tile_pool(name="vt", bufs=3) as vtp, \
                 tc.tile_pool(name="ho", bufs=2) as hop, \
                 tc.tile_pool(name="gps", bufs=3, space="PSUM") as gps, \
                 tc.tile_pool(name="qps", bufs=1, space="PSUM") as qps:
                for g in range(4):           # pair groups (2 pairs each)
                    N2P = gbp.tile([128, 1024], F32, tag="N2P", name="N2P")
                    RZs = []
                    for k in range(2):
                        u = 2 * g + k
                        xh = XH[u]
                        PRZ = gps.tile([128, 1024], F32, tag="g", name="PRZ")
                        for h in range(2):
                            nc.tensor.matmul(PRZ[:, 512 * h:512 * (h + 1)],
                                             C['CRZ'][:],
                                             xh[:, 512 * h:512 * (h + 1)],
                                             start=True, stop=True)
                        RZS = gbp.tile([128, 1024], F32, tag="RZS", bufs=3,
                                       name="RZS")
                        nc.scalar.activation(RZS[:], PRZ[:], AF.Sigmoid,
                                             bias=C['brz'][:])
                        RZs.append(RZS)
                        PGI = gps.tile([64, 1024], F32, tag="g", name="PGI")
                        PGH = gps.tile([64, 1024], F32, tag="g", name="PGH")
                        for h in range(2):
                            nc.tensor.matmul(PGI[:, 512 * h:512 * (h + 1)],
                                             C['CNI'][:],
                                             xh[0:64, 512 * h:512 * (h + 1)],
                                             start=True, stop=True)
                            nc.tensor.matmul(PGH[:, 512 * h:512 * (h + 1)],
                                             C['CNH'][64:128, :],
                                             xh[64:128, 512 * h:512 * (h + 1)],
                                             start=True, stop=True)
                        V1 = vtp.tile([64, 1024], F32, tag="v1", bufs=2,
                                      name="V1")
                        nc.vector.scalar_tensor_tensor(V1[:], PGH[:],
                                                       C['bhhn'][:],
                                                       RZS[0:64, :],
                                                       OP.add, OP.mult)
                        V3 = vtp.tile([64, 1024], F32, tag="v3", bufs=2,
                                      name="V3")
                        nc.vector.tensor_tensor(V3[:], V1[:], PGI[:], OP.add)
                        nc.scalar.activation(N2P[64 * k:64 * k + 64, :],
                                             V3[:], AF.Tanh,
                                             bias=C['bihn'][:])
                    H64P = gbp.tile([128, 1024], F32R, tag="H64P", bufs=2,
                                    name="H64P")
                    Z64P = gbp.tile([128, 1024], F32, tag="Z64P", bufs=2,
                                    name="Z64P")
                    for k in range(2):
                        u = 2 * g + k
                        nc.gpsimd.dma_start(
                            out=H64P[64 * k:64 * k + 64, :],
                            in_=hTD[:, 1024 * u:1024 * (u + 1)])
                        nc.gpsimd.dma_start(out=Z64P[64 * k:64 * k + 64, :],
                                            in_=RZs[k][64:128, :])
                    V4 = vtp.tile([128, 1024], F32, tag="vt", name="V4")
                    nc.vector.tensor_tensor(V4[:], H64P[:].bitcast(F32),
                                            N2P[:], OP.subtract)
                    V5 = vtp.tile([128, 1024], F32, tag="vt", name="V5")
                    nc.vector.tensor_tensor(V5[:], Z64P[:], V4[:], OP.mult)
                    HOP = hop.tile([128, 1024], F32R, tag="HO", name="HOP")
                    nc.vector.tensor_tensor(HOP[:], N2P[:], V5[:], OP.add)
                    for k in range(2):
                        u = 2 * g + k
                        nc.gpsimd.dma_start(
                            out=hoT[:, 1024 * u:1024 * (u + 1)],
                            in_=HOP[64 * k:64 * k + 64, :])
                        Q2 = qps.tile([5, 1024], F32, tag="q", name="Q2")
                        for h in range(2):
                            nc.tensor.matmul(
                                Q2[:, 512 * h:512 * (h + 1)],
                                C['CF2'][64 * k:64 * k + 64, :],
                                HOP[64 * k:64 * k + 64,
                                    512 * h:512 * (h + 1)],
                                start=True, stop=True)
                        QS = gbp.tile([5, 1024], F32, tag="QS", bufs=2,
                                      name="QS")
                        nc.vector.tensor_scalar_add(QS[:], Q2[:], C['bq'][:])
                        nc.gpsimd.dma_start(
                            out=qT[:, 1024 * u:1024 * (u + 1)], in_=QS[:])

    nc.compile()
    return nc


def _pack_consts(consts):
    """Pack all constant arrays into two [128, N] arrays (f32r-bound ones and
    fp32 biases) so startup is 2 DMAs instead of 26. Returns (pkr, pkb,
    shapes-dict with slice maps)."""
    BIASN = {'bP', 'b1a', 'b1b', 'b2', 'b3', 'bf1', 'brz', 'bihn', 'bhhn',
             'bq'}
    rmap, bmap = {}, {}
    rcols = bcols = 0
    for name, a in consts.items():
        if name in BIASN:
            bmap[name] = (0, a.shape[0], bcols, bcols + a.shape[1])
            bcols += a.shape[1]
        else:
            rmap[name] = (0, a.shape[0], rcols, rcols + a.shape[1])
            rcols += a.shape[1]
    pkr = np.zeros((128, rcols), np.float32)
    pkb = np.zeros((128, bcols), np.float32)
    for name, a in consts.items():
        if name in BIASN:
            p0, p1, c0, c1 = bmap[name]
            pkb[p0:p1, c0:c1] = a
        else:
            p0, p1, c0, c1 = rmap[name]
            pkr[p0:p1, c0:c1] = a
    return pkr, pkb, rmap, bmap


def kernel(**inputs):
    from concourse.bass_utils import run_bass_kernel_spmd

    obs = np.asarray(inputs['obs'], np.float32)
    hid = np.asarray(inputs['hidden_state'], np.float32)
    consts = _constants({k: np.asarray(v, np.float32)
                         for k, v in inputs.items()
                         if k not in ('obs', 'hidden_state')})
    pkr, pkb, rmap, bmap = _pack_consts(consts)
    cshapes = {'pkr': pkr.shape, 'pkb': pkb.shape, 'rmap': rmap, 'bmap': bmap}

    if 'nc' not in _CACHE:
        _CACHE['nc'] = _build_program(cshapes)
    nc = _CACHE['nc']

    perm = [(4 + r) % 80 for r in range(80)] + [80, 81, 82, 83, 84]
    obsT = np.zeros((96, B), np.float32)                  # rows 85:96 stay 0
    obsT[0:85] = obs.T[perm]
    hT = np.ascontiguousarray(hid.T)                      # [64, B]

    in_maps = []
    for c in range(NCORES):
        cols = slice(c * BL, (c + 1) * BL)
        m = {'obsx': np.ascontiguousarray(obsT[:, cols]),
             'hTD': np.ascontiguousarray(hT[:, cols]),
             'pkr': pkr, 'pkb': pkb}
        in_maps.append(m)

    res = run_bass_kernel_spmd(nc, in_maps, list(range(NCORES)))

    q = np.empty((B, 5), np.float32)
    h = np.empty((B, 64), np.float32)
    for c, r in enumerate(res.results):
        cols = slice(c * BL, (c + 1) * BL)
        q[cols] = r['qT'].T
        h[cols] = r['hoT'].T
    return q, h
